# revision 1
# baseline (speedup 1.0000x reference)
"""CSS2D (cross selective-scan 2D) Trainium2 kernel.

Sharding: 8 cores = batch(2) x scan-direction(4). Each core runs the full
pipeline for its (b, k) in the direction's own time order; direction
permutations are applied host-side to the inputs (and to the depthwise-conv
taps, which commute with grid transpose/reversal), so all 8 cores execute one
uniform SPMD program. The 4-direction merge is an AllGather within each
b-group followed by on-chip unpermute-and-add, LayerNorm, gating and the
output projection (computed redundantly per group; core 4b's output is used).
"""
import numpy as np
from contextlib import ExitStack

import concourse.bacc as bacc
import concourse.bass as bass
import concourse.mybir as mybir
import concourse.tile as tile
from concourse.bass_utils import run_bass_kernel_spmd

F32 = mybir.dt.float32
AF = mybir.ActivationFunctionType
OP = mybir.AluOpType

B_, HH, WW = 2, 32, 32
L = HH * WW                    # 1024
DM, DIN, N, R, K = 192, 384, 16, 12, 4
CDBL = R + 2 * N               # 44
NDT = DIN // 128               # 3 d-tiles
NG = 4                         # n-group size
NGRP = N // NG                 # 4 groups
NCORES = 8
PAD = 34 * 34                  # padded conv plane

_cache = {}


def _perm(k):
    t = np.arange(L)
    if k == 0:
        return t
    if k == 1:
        return (t % 32) * 32 + t // 32
    if k == 2:
        return 1023 - t
    return _perm(1)[1023 - t]


def _build_nc(stages=4, repeat=1):
    nc = bacc.Bacc(None, target_bir_lowering=False)

    def din(name, shape):
        return nc.declare_dram_parameter(name, list(shape), F32, isOutput=False)

    xT = din("xT", (DM, L))          # sigma_k-permuted x^T (this core's b)
    xcT = din("xcT", (DM, L))        # sigma_k-permuted x_cross^T
    xTc = din("xTc", (DM, L))        # common-order x^T (for z)
    wxpT = din("wxpT", (DM, DIN))
    wzT = din("wzT", (DM, DIN))
    wxcT = din("wxcT", (DM, DIN))
    wconv = din("wconv", (128, 9 * NDT * 128))   # diag blocks, col m=j*3+i
    convb = din("convb", (128, NDT))
    xprojT = din("xprojT", (128, NDT * 128))     # col-block i = W_k.T rows (scattered)
    dtwT = din("dtwT", (R, DIN))
    dtbias = din("dtbias", (128, NDT))
    Amat = din("Amat", (128, NDT * N))
    Dvec = din("Dvec", (128, NDT))
    onesm = din("onesm", (128, 128))             # 1/384
    gamma = din("gamma", (128, NDT))
    beta = din("beta", (128, NDT))
    outprojT = din("outprojT", (128, NDT * DM))  # col-block i = W_out.T rows
    nsel = din("nsel", (N, N * 128))             # selector lhsT per n
    epsc = din("epsc", (128, 1))                 # layernorm eps

    out_d = nc.declare_dram_parameter("out", [L, DM], F32, isOutput=True)

    with ExitStack() as ctx:
        tc = ctx.enter_context(tile.TileContext(nc))
        wpool = ctx.enter_context(tc.tile_pool(name="w", bufs=1))
        rpool = ctx.enter_context(tc.tile_pool(name="r", bufs=1))
        tpool = ctx.enter_context(tc.tile_pool(name="t", bufs=4))
        spool = ctx.enter_context(tc.tile_pool(name="s", bufs=1))
        apool = ctx.enter_context(tc.tile_pool(name="a", bufs=2))
        obpool = ctx.enter_context(tc.tile_pool(name="obp", bufs=2))
        ps_bc = ctx.enter_context(tc.tile_pool(name="psc", bufs=2, space="PSUM"))
        dram = ctx.enter_context(tc.tile_pool(name="dram", bufs=1, space="DRAM"))

        def tmp(shape=(128, L), tag="tmp"):
            return tpool.tile(list(shape), F32, tag=tag, name=tag)

        def big(shape, tag):  # 16KB scratch slots
            return spool.tile(list(shape), F32, tag=tag, name=tag)

        # ---- load weights/constants (persistent)
        def wload(src, shape, tag, col0=0, row0=0):
            t = wpool.tile(list(shape), F32, tag=tag, name=tag)
            nc.sync.dma_start(t[:], src[row0:row0 + shape[0], :])
            return t

        wxpT0 = wload(wxpT, (128, DIN), "wxpT0")
        wxpT1 = wload(wxpT, (64, DIN), "wxpT1", row0=128)
        wzT0 = wload(wzT, (128, DIN), "wzT0")
        wzT1 = wload(wzT, (64, DIN), "wzT1", row0=128)
        wxcT0 = wload(wxcT, (128, DIN), "wxcT0")
        wxcT1 = wload(wxcT, (64, DIN), "wxcT1", row0=128)
        wconv_s = wload(wconv, (128, 9 * NDT * 128), "wconv")
        convb_s = wload(convb, (128, NDT), "convb")
        xproj_s = wload(xprojT, (128, NDT * 128), "xproj")
        dtw_s = wload(dtwT, (R, DIN), "dtw")
        dtb_s = wload(dtbias, (128, NDT), "dtb")
        A_s = wload(Amat, (128, NDT * N), "Amat")
        D_s = wload(Dvec, (128, NDT), "Dvec")
        ones_s = wload(onesm, (128, 128), "ones")
        g_s = wload(gamma, (128, NDT), "gamma")
        bta_s = wload(beta, (128, NDT), "beta")
        wout_s = wload(outprojT, (128, NDT * DM), "wout")
        nsel_s = wload(nsel, (N, N * 128), "nsel")
        eps_s = wload(epsc, (128, 1), "epsc")

        for _rep in range(repeat):
            # ---- residents
            xp_pad = [rpool.tile([128, PAD], F32, tag=f"xp_pad{i}", name=f"xp_pad{i}")
                      for i in range(NDT)]
            xcs = [rpool.tile([128, L], F32, tag=f"xcs{i}", name=f"xcs{i}")
                   for i in range(NDT)]
            delta = [rpool.tile([128, L], F32, tag=f"delta{i}", name=f"delta{i}")
                     for i in range(NDT)]
            uu = [rpool.tile([128, L], F32, tag=f"u{i}", name=f"u{i}")
                  for i in range(NDT)]
            yac = [rpool.tile([128, L], F32, tag=f"yac{i}", name=f"yac{i}")
                   for i in range(NDT)]
            xdblB = rpool.tile([N, L], F32, tag="xdblB", name="xdblB")
            xdblC = rpool.tile([N, L], F32, tag="xdblC", name="xdblC")

            for i in range(NDT):
                nc.gpsimd.memset(xp_pad[i][:], 0.0)

            # ---- input loads (transient slots)
            xT0 = tmp(tag="tmp")
            nc.sync.dma_start(xT0[:], xT[0:128, :])
            xT1 = tmp((64, L), tag="tmp")
            nc.sync.dma_start(xT1[:], xT[128:192, :])
            xcT0 = tmp(tag="tmp")
            nc.sync.dma_start(xcT0[:], xcT[0:128, :])
            xcT1 = tmp((64, L), tag="tmp")
            nc.sync.dma_start(xcT1[:], xcT[128:192, :])

            def proj_mm(w0, w1, r0, r1, i, half):
                ps = ps_bc.tile([128, 512], F32, tag="bc", name="pb")
                cs = slice(i * 128, (i + 1) * 128)
                hs = slice(half * 512, (half + 1) * 512)
                nc.tensor.matmul(ps[:], w0[:, cs], r0[:, hs], start=True, stop=False)
                nc.tensor.matmul(ps[:], w1[:, cs], r1[:, hs], start=False, stop=True)
                return ps

            # xp projection into padded plane
            for i in range(NDT):
                for half in range(2):
                    ps = proj_mm(wxpT0, wxpT1, xT0, xT1, i, half)
                    dst = xp_pad[i][:].rearrange("p (h w) -> p h w", h=34)
                    h0 = 1 + 16 * half
                    nc.scalar.copy(dst[:, h0:h0 + 16, 1:33],
                                   ps[:].rearrange("p (h w) -> p h w", h=16))
            # xc projection
            for i in range(NDT):
                for half in range(2):
                    ps = proj_mm(wxcT0, wxcT1, xcT0, xcT1, i, half)
                    nc.scalar.copy(xcs[i][:, half * 512:(half + 1) * 512], ps[:])

            # ---- depthwise conv 3x3 (PE diag-matmuls) + bias + silu
            xh = [xp_pad[i][:, 0:L] for i in range(NDT)]
            for i in range(NDT):
                psc = ps_bc.tile([128, 1024], F32, tag="bc", name="psconv")
                pad3 = xp_pad[i][:].rearrange("p (h w) -> p h w", h=34)
                for j in range(9):
                    oh, ow = j // 3, j % 3
                    wsl = wconv_s[:, (j * NDT + i) * 128:(j * NDT + i) * 128 + 128]
                    for half in range(2):
                        h0 = oh + 16 * half
                        win = pad3[:, h0:h0 + 16, ow:ow + 32]
                        nc.tensor.matmul(psc[:, half * 512:(half + 1) * 512],
                                         wsl, win, start=(j == 0), stop=(j == 8))
                pa = tmp()
                nc.scalar.activation(pa[:], psc[:], AF.Identity,
                                     bias=convb_s[:, i:i + 1], scale=1.0)
                sg = tmp()
                nc.scalar.activation(sg[:], pa[:], AF.Sigmoid)
                nc.vector.tensor_tensor(xh[i], pa[:], sg[:], OP.mult)

            # ---- x_dbl = W_k @ xsc   [44, L]
            psx = ps_bc.tile([128, 1024], F32, tag="bc", name="pb")
            for half in range(2):
                hs = slice(half * 512, (half + 1) * 512)
                for i in range(NDT):
                    nc.tensor.matmul(psx[:, hs],
                                     xproj_s[:, i * 128:(i + 1) * 128],
                                     xcs[i][:, hs], start=(i == 0), stop=(i == NDT - 1))
            xdbl_dt = tmp((R, L), tag="tmp")
            nc.scalar.copy(xdbl_dt[:], psx[0:R, :])
            nc.scalar.copy(xdblB[:], psx[32:32 + N, :])
            nc.scalar.copy(xdblC[:], psx[64:64 + N, :])

            # ---- dts -> delta = softplus(dts + bias), u = delta*xs, y_acc = D*xs
            for i in range(NDT):
                psd = ps_bc.tile([128, 1024], F32, tag="bc", name="pb")
                for half in range(2):
                    hs = slice(half * 512, (half + 1) * 512)
                    nc.tensor.matmul(psd[:, hs], dtw_s[:, i * 128:(i + 1) * 128],
                                     xdbl_dt[:, hs], start=True, stop=True)
                et = tmp()
                nc.scalar.activation(et[:], psd[:], AF.Exp,
                                     bias=dtb_s[:, i:i + 1], scale=1.0)
                nc.scalar.activation(delta[i][:], et[:], AF.Ln, bias=1.0)
                nc.vector.tensor_tensor(uu[i][:], delta[i][:], xh[i], OP.mult)
                nc.vector.tensor_scalar_mul(yac[i][:], xh[i], D_s[:, i:i + 1])

            # ---- scan phase: n-pair groups outer (B/C broadcast reused across d-tiles)
            NG2 = 2
            for g in range(N // NG2 if stages >= 2 else 0):
                psB = ps_bc.tile([128, NG2, L], F32, tag="bc", name="psB")
                psC = ps_bc.tile([128, NG2, L], F32, tag="bc", name="psC")
                for j in range(NG2):
                    n = g * NG2 + j
                    for half in range(2):
                        hs = slice(half * 512, (half + 1) * 512)
                        nc.tensor.matmul(psB[:, j, hs],
                                         nsel_s[:, n * 128:(n + 1) * 128],
                                         xdblB[:, hs], start=True, stop=True)
                        nc.tensor.matmul(psC[:, j, hs],
                                         nsel_s[:, n * 128:(n + 1) * 128],
                                         xdblC[:, hs], start=True, stop=True)
                for i in range(NDT):
                    a_t = apool.tile([128, NG2, L], F32, tag="a_t", name="a_t")
                    for j in range(NG2):
                        n = g * NG2 + j
                        nc.scalar.activation(a_t[:, j, :], delta[i][:], AF.Exp,
                                             scale=A_s[:, i * N + n:i * N + n + 1])
                    nc.gpsimd.memset(a_t[:, :, 0:1], 0.0)

                    b_t = big([128, NG2, L], "b_t")
                    u_b = uu[i][:, None, :]
                    a0, a1 = bass.broadcast_tensor_aps(u_b, psB[:])
                    nc.vector.tensor_tensor(b_t[:], a0, a1, OP.mult)

                    h_t = big([128, NG2, L], "h_t")
                    nc.vector.tensor_tensor_scan(
                        h_t[:].rearrange("p n t -> p (n t)"),
                        a_t[:].rearrange("p n t -> p (n t)"),
                        b_t[:].rearrange("p n t -> p (n t)"),
                        0.0, OP.mult, OP.add)

                    hc_t = apool.tile([128, NG2, L], F32, tag="a_t", name="hc_t")
                    nc.vector.tensor_tensor(hc_t[:], h_t[:], psC[:], OP.mult)

                    ty = tmp()
                    nc.vector.tensor_reduce(ty[:], hc_t[:].rearrange("p n t -> p t n"),
                                            mybir.AxisListType.X, OP.add)
                    nc.vector.tensor_tensor(yac[i][:], yac[i][:], ty[:], OP.add)

            # ---- merge across directions (AllGather within b-group)
            do_merge = stages >= 3
            ybounce = dram.tile([DIN, L], F32, tag="ybounce", name="ybounce")
            ygather = dram.tile([K * DIN, L], F32, tag="ygather", name="ygather")
            for i in range(NDT if do_merge else 0):
                nc.sync.dma_start(ybounce[i * 128:(i + 1) * 128, :], yac[i][:])
            if do_merge:
             nc.gpsimd.collective_compute(
                "AllGather", OP.bypass,
                replica_groups=[[0, 1, 2, 3], [4, 5, 6, 7]],
                ins=[ybounce[:].opt()], outs=[ygather[:].opt()])

            yc = []
            for i in range(NDT if do_merge else 0):
                sl = [tmp(tag="tmp") for _ in range(K)]
                for k in range(K):
                    nc.sync.dma_start(sl[k][:], ygather[k * DIN + i * 128:
                                                       k * DIN + (i + 1) * 128, :])
                r1 = sl[1][:].rearrange("p (w h) -> p h w", w=32)
                r2 = sl[2][:, ::-1]
                r3 = sl[3][:, ::-1].rearrange("p (w h) -> p h w", w=32)
                t01 = big([128, L], "b_t")
                nc.vector.tensor_tensor(t01[:].rearrange("p (h w) -> p h w", h=32),
                                        sl[0][:].rearrange("p (h w) -> p h w", h=32),
                                        r1, OP.add)
                t23 = big([128, L], "h_t")
                nc.vector.tensor_tensor(t23[:].rearrange("p (h w) -> p h w", h=32),
                                        r2.rearrange("p (h w) -> p h w", h=32),
                                        r3, OP.add)
                yci = rpool.tile([128, L], F32, tag=f"xcs{i}", name=f"yc{i}")
                nc.vector.tensor_tensor(yci[:], t01[:], t23[:], OP.add)
                yc.append(yci)

            # ---- z projection + silu (post-merge; reuses freed slots)
            if not do_merge:
                yc = yac
            do_ln = stages >= 4
            xTc0 = big([128, L], "b_t")
            nc.sync.dma_start(xTc0[:], xTc[0:128, :])
            xTc1 = big([64, L], "h_t")
            nc.sync.dma_start(xTc1[:], xTc[128:192, :])
            zsl = []
            for i in range(NDT):
                zt = tmp()
                for half in range(2):
                    ps = proj_mm(wzT0, wzT1, xTc0, xTc1, i, half)
                    nc.scalar.copy(zt[:, half * 512:(half + 1) * 512], ps[:])
                zsg = tmp()
                nc.scalar.activation(zsg[:], zt[:], AF.Sigmoid)
                zsi = rpool.tile([128, L], F32, tag=f"delta{i}", name=f"zs{i}")
                nc.vector.tensor_tensor(zsi[:], zt[:], zsg[:], OP.mult)
                zsl.append(zsi)

            # ---- LayerNorm stats (ones-matmul broadcast)
            ysq = []
            for i in range(NDT if stages >= 3.5 else 0):
                q = rpool.tile([128, L], F32, tag=f"xp_pad{i}", name=f"ysq{i}")
                nc.scalar.activation(q[:], yc[i][:], AF.Square)
                ysq.append(q)
            psmu = ps_bc.tile([128, 1024], F32, tag="bc", name="pb")
            psms = ps_bc.tile([128, 1024], F32, tag="bc", name="pb")
            for half in range(2 if stages >= 3.5 else 0):
                hs = slice(half * 512, (half + 1) * 512)
                for i in range(NDT):
                    nc.tensor.matmul(psmu[:, hs], ones_s[:], yc[i][:, hs],
                                     start=(i == 0), stop=(i == NDT - 1))
                for i in range(NDT):
                    nc.tensor.matmul(psms[:, hs], ones_s[:], ysq[i][:, hs],
                                     start=(i == 0), stop=(i == NDT - 1))
            mu_sb = tmp()
            if stages >= 3.5:
                nc.scalar.copy(mu_sb[:], psmu[:])
            else:
                nc.vector.memset(mu_sb[:], 0.0)
            inv = tmp()
            if do_ln:
                musq = tmp()
                nc.vector.tensor_tensor(musq[:], mu_sb[:], mu_sb[:], OP.mult)
                vart = tmp()
                nc.vector.tensor_tensor(vart[:], psms[:], musq[:], OP.subtract)
                lnv = tmp()
                nc.scalar.activation(lnv[:], vart[:], AF.Ln, bias=eps_s[:, 0:1])
                nc.scalar.activation(inv[:], lnv[:], AF.Exp, scale=-0.5)
            else:
                nc.vector.memset(inv[:], 1.0)

            # ---- normalize + gate + out projection
            yg = []
            for i in range(NDT):
                d1 = big([128, L], "b_t")
                nc.vector.tensor_tensor(d1[:], yc[i][:], mu_sb[:], OP.subtract)
                d2 = big([128, L], "h_t")
                nc.vector.tensor_tensor(d2[:], d1[:], inv[:], OP.mult)
                d3 = rpool.tile([128, L], F32, tag=f"xcs{i}", name=f"d3_{i}")
                nc.scalar.activation(d3[:], d2[:], AF.Identity,
                                     bias=bta_s[:, i:i + 1], scale=g_s[:, i:i + 1])
                ygi = rpool.tile([128, L], F32, tag=f"yac{i}", name=f"yg{i}")
                nc.vector.tensor_tensor(ygi[:], d3[:], zsl[i][:], OP.mult)
                yg.append(ygi)

            for c in range(8):
                pso = ps_bc.tile([128, DM], F32, tag="bc", name="pb")
                for i in range(NDT):
                    nc.tensor.matmul(pso[:], yg[i][:, c * 128:(c + 1) * 128],
                                     wout_s[:, i * DM:(i + 1) * DM],
                                     start=(i == 0), stop=(i == NDT - 1))
                ob = obpool.tile([128, DM], F32, tag="ob", name="ob")
                nc.scalar.copy(ob[:], pso[:])
                nc.sync.dma_start(out_d[c * 128:(c + 1) * 128, :], ob[:])

    nc.compile()
    return nc


def _prep_maps(inputs):
    x = np.asarray(inputs["x"], np.float32)
    x_cross = np.asarray(inputs["x_cross"], np.float32)
    in_proj_w = np.asarray(inputs["in_proj_w"], np.float32)
    in_proj_cross_w = np.asarray(inputs["in_proj_cross_w"], np.float32)
    conv_w = np.asarray(inputs["conv_w"], np.float32)
    conv_b = np.asarray(inputs["conv_b"], np.float32)
    x_proj_weight = np.asarray(inputs["x_proj_weight"], np.float32)
    dt_projs_weight = np.asarray(inputs["dt_projs_weight"], np.float32)
    dt_projs_bias = np.asarray(inputs["dt_projs_bias"], np.float32)
    A_logs = np.asarray(inputs["A_logs"], np.float32)
    Ds = np.asarray(inputs["Ds"], np.float32)
    out_norm_g = np.asarray(inputs["out_norm_g"], np.float32)
    out_norm_b = np.asarray(inputs["out_norm_b"], np.float32)
    out_proj_w = np.asarray(inputs["out_proj_w"], np.float32)

    W_xp = in_proj_w[:DIN]
    W_z = in_proj_w[DIN:2 * DIN]
    A_full = (-np.exp(A_logs)).reshape(K, DIN, N)
    Ds_k = Ds.reshape(K, DIN)

    def fold3(v):  # [384] -> [128, 3]
        return np.ascontiguousarray(v.reshape(NDT, 128).T)

    common = {
        "wxpT": np.ascontiguousarray(W_xp.T),
        "wzT": np.ascontiguousarray(W_z.T),
        "wxcT": np.ascontiguousarray(in_proj_cross_w.T),
        "convb": fold3(conv_b),
        "onesm": np.full((128, 128), 1.0 / DIN, np.float32),
        "gamma": fold3(out_norm_g),
        "beta": fold3(out_norm_b),
        "outprojT": np.ascontiguousarray(
            out_proj_w.T.reshape(NDT, 128, DM).transpose(1, 0, 2).reshape(128, NDT * DM)),
        "epsc": np.full((128, 1), 1e-5, np.float32),
    }
    nsel = np.zeros((N, N * 128), np.float32)
    for n in range(N):
        nsel[n, n * 128:(n + 1) * 128] = 1.0
    common["nsel"] = nsel

    in_maps = []
    for c in range(NCORES):
        b, k = c // 4, c % 4
        p = _perm(k)
        xb = x[b].reshape(L, DM)
        xcb = x_cross[b].reshape(L, DM)
        w = conv_w[:, 0]  # [384, 3, 3]
        if k == 0:
            wk = w
        elif k == 1:
            wk = w.transpose(0, 2, 1)
        elif k == 2:
            wk = w[:, ::-1, ::-1]
        else:
            wk = w.transpose(0, 2, 1)[:, ::-1, ::-1]
        wconv = np.zeros((128, 9 * NDT * 128), np.float32)
        for j in range(9):
            for i in range(NDT):
                m = j * NDT + i
                dgv = np.ascontiguousarray(wk[i * 128:(i + 1) * 128, j // 3, j % 3])
                wconv[:, m * 128:m * 128 + 128] = np.diag(dgv)
        xp_w = x_proj_weight[k]  # [44, 384]
        xp_scat = np.zeros((DIN, 128), np.float32)   # lhsT cols = out partition
        xp_scat[:, 0:R] = xp_w[0:R].T
        xp_scat[:, 32:32 + N] = xp_w[R:R + N].T
        xp_scat[:, 64:64 + N] = xp_w[R + N:R + 2 * N].T
        xproj = np.ascontiguousarray(
            xp_scat.reshape(NDT, 128, 128).transpose(1, 0, 2).reshape(128, NDT * 128))
        Am = np.ascontiguousarray(
            A_full[k].reshape(NDT, 128, N).transpose(1, 0, 2).reshape(128, NDT * N))
        m = dict(common)
        m.update({
            "xT": np.ascontiguousarray(xb[p].T),
            "xcT": np.ascontiguousarray(xcb[p].T),
            "xTc": np.ascontiguousarray(xb.T),
            "wconv": wconv,
            "xprojT": xproj,
            "dtwT": np.ascontiguousarray(dt_projs_weight[k].T),
            "dtbias": fold3(dt_projs_bias[k]),
            "Amat": Am,
            "Dvec": fold3(Ds_k[k]),
        })
        in_maps.append(m)
    return in_maps


def kernel(**inputs):
    if "nc" not in _cache:
        _cache["nc"] = _build_nc()
    nc = _cache["nc"]
    in_maps = _prep_maps(inputs)
    res = run_bass_kernel_spmd(nc, in_maps, core_ids=list(range(NCORES)))
    out = np.zeros((B_, L, DM), np.float32)
    out[0] = res.results[0]["out"]
    out[1] = res.results[4]["out"]
    return out.reshape(B_, HH, WW, DM)



# revision 13
# speedup vs baseline: 1.6343x; 1.6343x over previous
"""CSS2D (cross selective-scan 2D) Trainium2 kernel.

Sharding: 8 cores = batch(2) x scan-direction(4). Each core runs the full
pipeline for its (b, k) in the direction's own time order; direction
permutations are applied host-side to the inputs (and to the depthwise-conv
taps, which commute with grid transpose/reversal), so all 8 cores execute one
uniform SPMD program. The 4-direction merge is a per-dtile bf16 AllGather
within each b-group (overlapped with the scan of the next dtile) followed by
on-chip unpermute-and-add, LayerNorm, gating and the output projection
(computed redundantly per group; core 4b's output is used).

All matmuls run in bf16 (PE fp32 is 4 cycles/row vs 1 for bf16); the scan
elementwise chain runs in bf16 (DVE tensor_tensor 2x mode, quad-grouped
states) with the selective-scan internal state and the y accumulator in fp32.
"""
import numpy as np
import ml_dtypes
from contextlib import ExitStack

import concourse.bacc as bacc
import concourse.bass as bass
import concourse.mybir as mybir
import concourse.tile as tile
from concourse.bass_utils import run_bass_kernel_spmd

F32 = mybir.dt.float32
BF16 = mybir.dt.bfloat16
AF = mybir.ActivationFunctionType
OP = mybir.AluOpType

B_, HH, WW = 2, 32, 32
L = HH * WW                    # 1024
DM, DIN, N, R, K = 192, 384, 16, 12, 4
NDT = DIN // 128               # 3 d-tiles
QN = 4                         # states per scan quad
NQ = N // QN                   # 4 quads
NCORES = 8
PAD = 34 * 34                  # padded conv plane
BF = ml_dtypes.bfloat16

_cache = {}


def _perm(k):
    t = np.arange(L)
    if k == 0:
        return t
    if k == 1:
        return (t % 32) * 32 + t // 32
    if k == 2:
        return 1023 - t
    return _perm(1)[1023 - t]


def _build_nc():
    nc = bacc.Bacc(None, target_bir_lowering=False)

    def din(name, shape, dt=BF16):
        return nc.declare_dram_parameter(name, list(shape), dt, isOutput=False)

    xT = din("xT", (DM, L))          # sigma_k-permuted x^T (this core's b)
    xcT = din("xcT", (DM, L))        # sigma_k-permuted x_cross^T
    xTc = din("xTc", (DM, L))        # common-order x^T (for z)
    wxpT = din("wxpT", (DM, DIN))
    wzT = din("wzT", (DM, DIN))
    wxcT = din("wxcT", (DM, DIN))
    wconv = din("wconv", (128, 9 * NDT * 128))   # diag blocks, col m=j*3+i
    convb = din("convb", (128, NDT), F32)
    xprojT = din("xprojT", (128, NDT * 128))     # col-block i = W_k.T rows (scattered)
    dtwT = din("dtwT", (R, DIN))
    dtbias = din("dtbias", (128, NDT), F32)
    Amat = din("Amat", (128, NDT * N), F32)
    Dvec = din("Dvec", (128, NDT), F32)
    onesm = din("onesm", (128, 128))             # 1.0; mean scale applied on scalar
    gamma = din("gamma", (128, NDT), F32)
    beta = din("beta", (128, NDT), F32)
    outprojT = din("outprojT", (128, NDT * DM))  # col-block i = W_out.T rows
    nsel = din("nsel", (N, N * 128))             # selector lhsT per n
    epsc = din("epsc", (128, 1), F32)            # layernorm eps

    out_d = nc.declare_dram_parameter("out", [L, DM], F32, isOutput=True)

    with ExitStack() as ctx:
        tc = ctx.enter_context(tile.TileContext(nc))
        wpool = ctx.enter_context(tc.tile_pool(name="w", bufs=1))
        rpool = ctx.enter_context(tc.tile_pool(name="r", bufs=1))
        tpool = ctx.enter_context(tc.tile_pool(name="t", bufs=2))
        iopool = ctx.enter_context(tc.tile_pool(name="io", bufs=4))
        bcpool = ctx.enter_context(tc.tile_pool(name="bcp", bufs=1))
        apool = ctx.enter_context(tc.tile_pool(name="a", bufs=2))
        hpool = ctx.enter_context(tc.tile_pool(name="h", bufs=1))
        spool = ctx.enter_context(tc.tile_pool(name="s", bufs=1))
        obpool = ctx.enter_context(tc.tile_pool(name="obp", bufs=2))
        ps_bc = ctx.enter_context(tc.tile_pool(name="psc", bufs=2, space="PSUM"))
        dram = ctx.enter_context(tc.tile_pool(name="dram", bufs=1, space="DRAM"))

        def tmp(shape=(128, L), tag="tmp", dt=BF16):
            pool = iopool if tag == "tmp" else tpool
            return pool.tile(list(shape), dt, tag=tag, name=tag)

        # ---- load weights/constants (persistent)
        def wload(src, shape, tag, row0=0, dt=BF16):
            t = wpool.tile(list(shape), dt, tag=tag, name=tag)
            nc.sync.dma_start(t[:], src[row0:row0 + shape[0], :])
            return t

        wxpT0 = wload(wxpT, (128, DIN), "wxpT0")
        wxpT1 = wload(wxpT, (64, DIN), "wxpT1", row0=128)
        wzT0 = wload(wzT, (128, DIN), "wzT0")
        wzT1 = wload(wzT, (64, DIN), "wzT1", row0=128)
        wxcT0 = wload(wxcT, (128, DIN), "wxcT0")
        wxcT1 = wload(wxcT, (64, DIN), "wxcT1", row0=128)
        wconv_s = wload(wconv, (128, 9 * NDT * 128), "wconv")
        convb_s = wload(convb, (128, NDT), "convb", dt=F32)
        xproj_s = wload(xprojT, (128, NDT * 128), "xproj")
        dtw_s = wload(dtwT, (R, DIN), "dtw")
        dtb_s = wload(dtbias, (128, NDT), "dtb", dt=F32)
        A_s = wload(Amat, (128, NDT * N), "Amat", dt=F32)
        D_s = wload(Dvec, (128, NDT), "Dvec", dt=F32)
        ones_s = wload(onesm, (128, 128), "ones")
        g_s = wload(gamma, (128, NDT), "gamma", dt=F32)
        bta_s = wload(beta, (128, NDT), "beta", dt=F32)
        wout_s = wload(outprojT, (128, NDT * DM), "wout")
        nsel_s = wload(nsel, (N, N * 128), "nsel")
        eps_s = wload(epsc, (128, 1), "epsc", dt=F32)

        # ---- residents
        xp_pad = [rpool.tile([128, PAD], BF16, tag=f"xp_pad{i}", name=f"xp_pad{i}")
                  for i in range(NDT)]
        delta = [rpool.tile([128, L], BF16, tag=f"delta{i}", name=f"delta{i}")
                 for i in range(NDT)]
        uu = [rpool.tile([128, L], BF16, tag=f"u{i}", name=f"u{i}")
              for i in range(NDT)]
        yac = [rpool.tile([128, L], F32, tag=f"yac{i}", name=f"yac{i}")
               for i in range(NDT)]
        xcs = [rpool.tile([128, L], BF16, tag=f"xcs{i}", name=f"xcs{i}")
               for i in range(NDT)]

        xdblB = rpool.tile([N, L], BF16, tag="xdblB", name="xdblB")
        xdblC = rpool.tile([N, L], BF16, tag="xdblC", name="xdblC")
        xdbl_dt = rpool.tile([R, L], BF16, tag="xdbl_dt", name="xdbl_dt")
        # B/C broadcast tiles, bf16, quad-grouped [128, QN, L]
        Bq = [bcpool.tile([128, QN, L], BF16, tag=f"Bq{q}", name=f"Bq{q}")
              for q in range(NQ)]
        Cq = [bcpool.tile([128, QN, L], BF16, tag=f"Cq{q}", name=f"Cq{q}")
              for q in range(NQ)]

        for i in range(NDT):
            nc.gpsimd.memset(xp_pad[i][:], 0.0)

        # ---- input loads (transient slots)
        xT0 = tmp(tag="tmp")
        nc.sync.dma_start(xT0[:], xT[0:128, :])
        xT1 = tmp((64, L), tag="tmp")
        nc.sync.dma_start(xT1[:], xT[128:192, :])
        xcT0 = tmp(tag="tmp")
        nc.sync.dma_start(xcT0[:], xcT[0:128, :])
        xcT1 = tmp((64, L), tag="tmp")
        nc.sync.dma_start(xcT1[:], xcT[128:192, :])

        def proj_mm(w0, w1, r0, r1, i, half):
            ps = ps_bc.tile([128, 512], F32, tag="pm", name="pm")
            cs = slice(i * 128, (i + 1) * 128)
            hs = slice(half * 512, (half + 1) * 512)
            nc.tensor.matmul(ps[:], w0[:, cs], r0[:, hs], start=True, stop=False)
            nc.tensor.matmul(ps[:], w1[:, cs], r1[:, hs], start=False, stop=True)
            return ps

        # xc projection (feeds x_dbl -> B/C/delta: done first)
        for i in range(NDT):
            for half in range(2):
                ps = proj_mm(wxcT0, wxcT1, xcT0, xcT1, i, half)
                nc.scalar.copy(xcs[i][:, half * 512:(half + 1) * 512], ps[:])

        # ---- x_dbl = W_k @ xsc   [44, L] scattered at rows 0:R / 32:48 / 64:80
        for half in range(2):
            psx = ps_bc.tile([128, 512], F32, tag="pm", name="px")
            hs = slice(half * 512, (half + 1) * 512)
            for i in range(NDT):
                nc.tensor.matmul(psx[:], xproj_s[:, i * 128:(i + 1) * 128],
                                 xcs[i][:, hs], start=(i == 0), stop=(i == NDT - 1))
            nc.scalar.copy(xdbl_dt[:, hs], psx[0:R, :])
            nc.scalar.copy(xdblB[:, hs], psx[32:32 + N, :])
            nc.scalar.copy(xdblC[:, hs], psx[64:64 + N, :])

        # ---- B/C broadcast to 128 partitions (PE) then park in SBUF bf16
        for n in range(N):
            q, j = n // QN, n % QN
            psB = ps_bc.tile([128, L], F32, tag="pn", name="psB")
            psC = ps_bc.tile([128, L], F32, tag="pn", name="psC")
            for half in range(2):
                hs = slice(half * 512, (half + 1) * 512)
                nc.tensor.matmul(psB[:, hs], nsel_s[:, n * 128:(n + 1) * 128],
                                 xdblB[:, hs], start=True, stop=True)
                nc.tensor.matmul(psC[:, hs], nsel_s[:, n * 128:(n + 1) * 128],
                                 xdblC[:, hs], start=True, stop=True)
            nc.scalar.copy(Bq[q][:, j, :], psB[:])
            nc.scalar.copy(Cq[q][:, j, :], psC[:])

        # ---- xp projection into padded conv plane
        for i in range(NDT):
            for half in range(2):
                ps = proj_mm(wxpT0, wxpT1, xT0, xT1, i, half)
                dst = xp_pad[i][:].rearrange("p (h w) -> p h w", h=34)
                h0 = 1 + 16 * half
                nc.scalar.copy(dst[:, h0:h0 + 16, 1:33],
                               ps[:].rearrange("p (h w) -> p h w", h=16))

        # ---- depthwise conv 3x3 (PE diag-matmuls) + bias + silu
        xh = [xp_pad[i][:, 0:L] for i in range(NDT)]
        for i in range(NDT):
            psc = ps_bc.tile([128, 1024], F32, tag="pn", name="psconv")
            pad3 = xp_pad[i][:].rearrange("p (h w) -> p h w", h=34)
            for j in range(9):
                oh, ow = j // 3, j % 3
                wsl = wconv_s[:, (j * NDT + i) * 128:(j * NDT + i) * 128 + 128]
                for half in range(2):
                    h0 = oh + 16 * half
                    win = pad3[:, h0:h0 + 16, ow:ow + 32]
                    nc.tensor.matmul(psc[:, half * 512:(half + 1) * 512],
                                     wsl, win, start=(j == 0), stop=(j == 8))
            pa = tmp(tag="pa")
            nc.scalar.activation(pa[:], psc[:], AF.Identity,
                                 bias=convb_s[:, i:i + 1], scale=1.0)
            sg = tmp(tag="sg")
            nc.scalar.activation(sg[:], pa[:], AF.Sigmoid)
            nc.vector.tensor_tensor(xh[i], pa[:], sg[:], OP.mult)

        # ---- dts -> delta = softplus(dts + bias), u = delta*xs, y_acc = D*xs
        for i in range(NDT):
            psd = ps_bc.tile([128, 1024], F32, tag="pn", name="psd")
            for half in range(2):
                hs = slice(half * 512, (half + 1) * 512)
                nc.tensor.matmul(psd[:, hs], dtw_s[:, i * 128:(i + 1) * 128],
                                 xdbl_dt[:, hs], start=True, stop=True)
            et = tmp(tag="pa")
            nc.scalar.activation(et[:], psd[:], AF.Exp,
                                 bias=dtb_s[:, i:i + 1], scale=1.0)
            nc.scalar.activation(delta[i][:], et[:], AF.Ln, bias=1.0)
            nc.vector.tensor_tensor(uu[i][:], delta[i][:], xh[i], OP.mult)
            nc.vector.tensor_scalar_mul(yac[i][:], xh[i], D_s[:, i:i + 1])

        # ---- scan: dtile outer (per-dtile AllGather overlaps next dtile)
        ybounce = [dram.tile([128, L], BF16, tag=f"ybounce{i}", name=f"ybounce{i}")
                   for i in range(NDT)]
        ygather = [dram.tile([K * 128, L], BF16, tag=f"ygather{i}", name=f"ygather{i}")
                   for i in range(NDT)]
        for i in range(NDT):
            for q in range(NQ):
                a_q = apool.tile([128, QN, L], BF16, tag="a_q", name=f"a{i}{q}")
                for j in range(QN):
                    n = q * QN + j
                    nc.scalar.activation(a_q[:, j, :], delta[i][:], AF.Exp,
                                         scale=A_s[:, i * N + n:i * N + n + 1])
                nc.gpsimd.memset(a_q[:, :, 0:1], 0.0)

                b_q = hpool.tile([128, QN, L], BF16, tag="b_q", name=f"b{i}{q}")
                a0, a1 = bass.broadcast_tensor_aps(uu[i][:, None, :], Bq[q][:])
                nc.vector.tensor_tensor(b_q[:], a0, a1, OP.mult)

                h_q = hpool.tile([128, QN, L], BF16, tag="h_q", name=f"h{i}{q}")
                nc.vector.tensor_tensor_scan(
                    h_q[:].rearrange("p n t -> p (n t)"),
                    a_q[:].rearrange("p n t -> p (n t)"),
                    b_q[:].rearrange("p n t -> p (n t)"),
                    0.0, OP.mult, OP.add)

                # hc into b_q (dead), pair-tree into h_q (dead), acc into yac
                nc.vector.tensor_tensor(b_q[:], h_q[:], Cq[q][:], OP.mult)
                nc.vector.tensor_tensor(h_q[:, 0:2, :], b_q[:, 0:2, :],
                                        b_q[:, 2:4, :], OP.add)
                nc.vector.tensor_tensor(h_q[:, 2, :], h_q[:, 0, :],
                                        h_q[:, 1, :], OP.add)
                nc.vector.tensor_tensor(yac[i][:], yac[i][:], h_q[:, 2, :], OP.add)

            ybc = tmp(tag="sg")
            nc.vector.tensor_copy(ybc[:], yac[i][:])
            nc.sync.dma_start(ybounce[i][:], ybc[:])
            nc.gpsimd.collective_compute(
                "AllGather", OP.bypass,
                replica_groups=[[0, 1, 2, 3], [4, 5, 6, 7]],
                ins=[ybounce[i][:].opt()], outs=[ygather[i][:].opt()])

        # ---- merge across directions (unpermute each slot, add)
        yc = []
        for i in range(NDT):
            sl = [tmp(tag="tmp") for _ in range(K)]
            for k in range(K):
                nc.sync.dma_start(sl[k][:], ygather[i][k * 128:(k + 1) * 128, :])
            r1 = sl[1][:].rearrange("p (w h) -> p h w", w=32)
            r2 = sl[2][:, ::-1]
            r3 = sl[3][:, ::-1].rearrange("p (w h) -> p h w", w=32)
            t01 = spool.tile([128, L], BF16, tag="sc1", name="t01")
            nc.vector.tensor_tensor(t01[:].rearrange("p (h w) -> p h w", h=32),
                                    sl[0][:].rearrange("p (h w) -> p h w", h=32),
                                    r1, OP.add)
            t23 = spool.tile([128, L], BF16, tag="sc2", name="t23")
            nc.vector.tensor_tensor(t23[:].rearrange("p (h w) -> p h w", h=32),
                                    r2.rearrange("p (h w) -> p h w", h=32),
                                    r3, OP.add)
            yci = rpool.tile([128, L], BF16, tag=f"xcs{i}", name=f"yc{i}")
            nc.vector.tensor_tensor(yci[:], t01[:], t23[:], OP.add)
            yc.append(yci)

        # ---- z projection + silu
        xTc0 = spool.tile([128, L], BF16, tag="xtc0", name="xTc0")
        nc.sync.dma_start(xTc0[:], xTc[0:128, :])
        xTc1 = spool.tile([64, L], BF16, tag="xtc1", name="xTc1")
        nc.sync.dma_start(xTc1[:], xTc[128:192, :])
        # zsil reuses delta's buffers (delta is dead once dtile i's scan a_t
        # activations have been issued)
        zsil = []
        for i in range(NDT):
            zt = tmp(tag="pa")
            for half in range(2):
                ps = proj_mm(wzT0, wzT1, xTc0, xTc1, i, half)
                nc.scalar.copy(zt[:, half * 512:(half + 1) * 512], ps[:])
            zsg = tmp(tag="sg")
            nc.scalar.activation(zsg[:], zt[:], AF.Sigmoid)
            zsi = rpool.tile([128, L], BF16, tag=f"delta{i}", name=f"zsil{i}")
            nc.vector.tensor_tensor(zsi[:], zt[:], zsg[:], OP.mult)
            zsil.append(zsi)

        # ---- LayerNorm stats (ones-matmul partition reduction, bf16 in, fp32 acc)
        ysq = []
        for i in range(NDT):
            sq = rpool.tile([128, L], BF16, tag=f"xp_pad{i}", name=f"ysq{i}")
            nc.scalar.activation(sq[:], yc[i][:], AF.Square)
            ysq.append(sq)
        psmu = ps_bc.tile([128, 1024], F32, tag="pn", name="psmu")
        psms = ps_bc.tile([128, 1024], F32, tag="pn", name="psms")
        for half in range(2):
            hs = slice(half * 512, (half + 1) * 512)
            for i in range(NDT):
                nc.tensor.matmul(psmu[:, hs], ones_s[:], yc[i][:, hs],
                                 start=(i == 0), stop=(i == NDT - 1))
            for i in range(NDT):
                nc.tensor.matmul(psms[:, hs], ones_s[:], ysq[i][:, hs],
                                 start=(i == 0), stop=(i == NDT - 1))
        mu_sb = spool.tile([128, L], F32, tag="mu", name="mu")
        nc.scalar.mul(mu_sb[:], psmu[:], 1.0 / DIN)
        musq = spool.tile([128, L], F32, tag="musq", name="musq")
        nc.vector.tensor_tensor(musq[:], mu_sb[:], mu_sb[:], OP.mult)
        ms_sb = spool.tile([128, L], F32, tag="vart", name="ms")
        nc.scalar.mul(ms_sb[:], psms[:], 1.0 / DIN)
        vart = spool.tile([128, L], F32, tag="d1", name="vart")
        nc.vector.tensor_tensor(vart[:], ms_sb[:], musq[:], OP.subtract)
        lnv = spool.tile([128, L], F32, tag="musq", name="lnv")
        nc.scalar.activation(lnv[:], vart[:], AF.Ln, bias=eps_s[:, 0:1])
        inv = spool.tile([128, L], F32, tag="vart", name="inv")
        nc.scalar.activation(inv[:], lnv[:], AF.Exp, scale=-0.5)

        # ---- normalize + gate + out projection
        yg = []
        for i in range(NDT):
            d1 = spool.tile([128, L], F32, tag="d1", name=f"d1_{i}")
            nc.vector.tensor_tensor(d1[:], yc[i][:], mu_sb[:], OP.subtract)
            d2 = spool.tile([128, L], F32, tag="d2", name=f"d2_{i}")
            nc.vector.tensor_tensor(d2[:], d1[:], inv[:], OP.mult)
            d3 = spool.tile([128, L], BF16, tag="sc1", name=f"d3_{i}")
            nc.scalar.activation(d3[:], d2[:], AF.Identity,
                                 bias=bta_s[:, i:i + 1], scale=g_s[:, i:i + 1])
            ygi = rpool.tile([128, L], BF16, tag=f"u{i}", name=f"yg{i}")
            nc.vector.tensor_tensor(ygi[:], d3[:], zsil[i][:], OP.mult)
            yg.append(ygi)

        for c in range(8):
            pso = ps_bc.tile([128, DM], F32, tag="pm", name="pso")
            for i in range(NDT):
                nc.tensor.matmul(pso[:], yg[i][:, c * 128:(c + 1) * 128],
                                 wout_s[:, i * DM:(i + 1) * DM],
                                 start=(i == 0), stop=(i == NDT - 1))
            ob = obpool.tile([128, DM], F32, tag="ob", name="ob")
            nc.scalar.copy(ob[:], pso[:])
            nc.sync.dma_start(out_d[c * 128:(c + 1) * 128, :], ob[:])

    nc.compile()
    return nc


def _prep_maps(inputs):
    x = np.asarray(inputs["x"], np.float32)
    x_cross = np.asarray(inputs["x_cross"], np.float32)
    in_proj_w = np.asarray(inputs["in_proj_w"], np.float32)
    in_proj_cross_w = np.asarray(inputs["in_proj_cross_w"], np.float32)
    conv_w = np.asarray(inputs["conv_w"], np.float32)
    conv_b = np.asarray(inputs["conv_b"], np.float32)
    x_proj_weight = np.asarray(inputs["x_proj_weight"], np.float32)
    dt_projs_weight = np.asarray(inputs["dt_projs_weight"], np.float32)
    dt_projs_bias = np.asarray(inputs["dt_projs_bias"], np.float32)
    A_logs = np.asarray(inputs["A_logs"], np.float32)
    Ds = np.asarray(inputs["Ds"], np.float32)
    out_norm_g = np.asarray(inputs["out_norm_g"], np.float32)
    out_norm_b = np.asarray(inputs["out_norm_b"], np.float32)
    out_proj_w = np.asarray(inputs["out_proj_w"], np.float32)

    W_xp = in_proj_w[:DIN]
    W_z = in_proj_w[DIN:2 * DIN]
    A_full = (-np.exp(A_logs)).reshape(K, DIN, N)
    Ds_k = Ds.reshape(K, DIN)

    def fold3(v):  # [384] -> [128, 3]
        return np.ascontiguousarray(v.reshape(NDT, 128).T)

    common = {
        "wxpT": np.ascontiguousarray(W_xp.T).astype(BF),
        "wzT": np.ascontiguousarray(W_z.T).astype(BF),
        "wxcT": np.ascontiguousarray(in_proj_cross_w.T).astype(BF),
        "convb": fold3(conv_b),
        "onesm": np.full((128, 128), 1.0, BF),
        "gamma": fold3(out_norm_g),
        "beta": fold3(out_norm_b),
        "outprojT": np.ascontiguousarray(
            out_proj_w.T.reshape(NDT, 128, DM).transpose(1, 0, 2)
            .reshape(128, NDT * DM)).astype(BF),
        "epsc": np.full((128, 1), 1e-5, np.float32),
    }
    nsel = np.zeros((N, N * 128), np.float32)
    for n in range(N):
        nsel[n, n * 128:(n + 1) * 128] = 1.0
    common["nsel"] = nsel.astype(BF)

    in_maps = []
    for c in range(NCORES):
        b, k = c // 4, c % 4
        p = _perm(k)
        xb = x[b].reshape(L, DM)
        xcb = x_cross[b].reshape(L, DM)
        w = conv_w[:, 0]  # [384, 3, 3]
        if k == 0:
            wk = w
        elif k == 1:
            wk = w.transpose(0, 2, 1)
        elif k == 2:
            wk = w[:, ::-1, ::-1]
        else:
            wk = w.transpose(0, 2, 1)[:, ::-1, ::-1]
        wconv = np.zeros((128, 9 * NDT * 128), np.float32)
        for j in range(9):
            for i in range(NDT):
                m = j * NDT + i
                dgv = np.ascontiguousarray(wk[i * 128:(i + 1) * 128, j // 3, j % 3])
                wconv[:, m * 128:m * 128 + 128] = np.diag(dgv)
        xp_w = x_proj_weight[k]  # [44, 384]
        xp_scat = np.zeros((DIN, 128), np.float32)   # lhsT cols = out partition
        xp_scat[:, 0:R] = xp_w[0:R].T
        xp_scat[:, 32:32 + N] = xp_w[R:R + N].T
        xp_scat[:, 64:64 + N] = xp_w[R + N:R + 2 * N].T
        xproj = np.ascontiguousarray(
            xp_scat.reshape(NDT, 128, 128).transpose(1, 0, 2).reshape(128, NDT * 128))
        Am = np.ascontiguousarray(
            A_full[k].reshape(NDT, 128, N).transpose(1, 0, 2).reshape(128, NDT * N))
        m = dict(common)
        m.update({
            "xT": np.ascontiguousarray(xb[p].T).astype(BF),
            "xcT": np.ascontiguousarray(xcb[p].T).astype(BF),
            "xTc": np.ascontiguousarray(xb.T).astype(BF),
            "wconv": wconv.astype(BF),
            "xprojT": xproj.astype(BF),
            "dtwT": np.ascontiguousarray(dt_projs_weight[k].T).astype(BF),
            "dtbias": fold3(dt_projs_bias[k]),
            "Amat": Am,
            "Dvec": fold3(Ds_k[k]),
        })
        in_maps.append(m)
    return in_maps


def kernel(**inputs):
    if "nc" not in _cache:
        _cache["nc"] = _build_nc()
    nc = _cache["nc"]
    in_maps = _prep_maps(inputs)
    res = run_bass_kernel_spmd(nc, in_maps, core_ids=list(range(NCORES)))
    out = np.zeros((B_, L, DM), np.float32)
    out[0] = res.results[0]["out"]
    out[1] = res.results[4]["out"]
    return out.reshape(B_, HH, WW, DM)


# revision 25
# speedup vs baseline: 1.9257x; 1.1783x over previous
"""CSS2D (cross selective-scan 2D) Trainium2 kernel.

Sharding: 8 cores = batch(2) x scan-direction(4). Each core runs the full
pipeline for its (b, k) in the direction's own time order; direction
permutations are applied host-side to the inputs (and to the depthwise-conv
taps, which commute with grid transpose/reversal), so all 8 cores execute one
uniform SPMD program. The 4-direction merge is a per-dtile bf16 AllGather
within each b-group (overlapped with the scan of the next dtile) followed by
on-chip unpermute-and-add, LayerNorm, gating and the output projection
(computed redundantly per group; core 4b's output is used).

All matmuls run in bf16 (PE fp32 is 4 cycles/row vs 1 for bf16); the scan
elementwise chain runs in bf16 (DVE tensor_tensor 2x mode, quad-grouped
states) with the selective-scan internal state and the y accumulator in fp32.
"""
import numpy as np
import ml_dtypes
from contextlib import ExitStack

import concourse.bacc as bacc
import concourse.bass as bass
import concourse.mybir as mybir
import concourse.tile as tile
from concourse.bass_utils import run_bass_kernel_spmd

F32 = mybir.dt.float32
BF16 = mybir.dt.bfloat16
AF = mybir.ActivationFunctionType
OP = mybir.AluOpType

B_, HH, WW = 2, 32, 32
L = HH * WW                    # 1024
DM, DIN, N, R, K = 192, 384, 16, 12, 4
NDT = DIN // 128               # 3 d-tiles
QN = 4                         # states per scan quad
NQ = N // QN                   # 4 quads
NCORES = 8
PAD = 34 * 34                  # padded conv plane
BF = ml_dtypes.bfloat16

_cache = {}


def _perm(k):
    t = np.arange(L)
    if k == 0:
        return t
    if k == 1:
        return (t % 32) * 32 + t // 32
    if k == 2:
        return 1023 - t
    return _perm(1)[1023 - t]


def _build_nc():
    nc = bacc.Bacc(None, target_bir_lowering=False)

    def din(name, shape, dt=BF16):
        return nc.declare_dram_parameter(name, list(shape), dt, isOutput=False)

    xT = din("xT", (DM, L))          # sigma_k-permuted x^T (this core's b)
    xcT = din("xcT", (DM, L))        # sigma_k-permuted x_cross^T
    xTc = din("xTc", (DM, L))        # common-order x^T (for z)
    wxpT = din("wxpT", (DM, DIN))
    wzT = din("wzT", (DM, DIN))
    wxcT = din("wxcT", (DM, DIN))
    wconv = din("wconv", (128, 9 * NDT * 128))   # diag blocks, col m=j*3+i
    convb = din("convb", (128, NDT), F32)
    xprojT = din("xprojT", (128, NDT * 128))     # col-block i = W_k.T rows (scattered)
    dtwT = din("dtwT", (R, DIN))
    dtbias = din("dtbias", (128, NDT), F32)
    Amat = din("Amat", (128, NDT * N), F32)
    Dvec = din("Dvec", (128, NDT), F32)
    onesm = din("onesm", (128, 128))             # 1.0; mean scale applied on scalar
    gamma = din("gamma", (128, NDT), F32)
    beta = din("beta", (128, NDT), F32)
    outprojT = din("outprojT", (128, NDT * DM))  # col-block i = W_out.T rows
    epsc = din("epsc", (128, 1), F32)            # layernorm eps

    out_d = nc.declare_dram_parameter("out", [L, DM], F32, isOutput=True)

    with ExitStack() as ctx:
        tc = ctx.enter_context(tile.TileContext(nc))
        wpool = ctx.enter_context(tc.tile_pool(name="w", bufs=1))
        rpool = ctx.enter_context(tc.tile_pool(name="r", bufs=1))
        tpool = ctx.enter_context(tc.tile_pool(name="t", bufs=2))
        iopool = ctx.enter_context(tc.tile_pool(name="io", bufs=4))
        bcpool = ctx.enter_context(tc.tile_pool(name="bcp", bufs=1))
        apool = ctx.enter_context(tc.tile_pool(name="a", bufs=2))
        hpool = ctx.enter_context(tc.tile_pool(name="h", bufs=1))
        spool = ctx.enter_context(tc.tile_pool(name="s", bufs=1))
        obpool = ctx.enter_context(tc.tile_pool(name="obp", bufs=2))
        ps_bc = ctx.enter_context(tc.tile_pool(name="psc", bufs=2, space="PSUM"))
        dram = ctx.enter_context(tc.tile_pool(name="dram", bufs=1, space="DRAM"))

        def tmp(shape=(128, L), tag="tmp", dt=BF16):
            pool = iopool if tag == "tmp" else tpool
            return pool.tile(list(shape), dt, tag=tag, name=tag)

        # ---- load weights/constants (persistent)
        def wload(src, shape, tag, row0=0, dt=BF16):
            t = wpool.tile(list(shape), dt, tag=tag, name=tag)
            nc.sync.dma_start(t[:], src[row0:row0 + shape[0], :])
            return t

        wxpT0 = wload(wxpT, (128, DIN), "wxpT0")
        wxpT1 = wload(wxpT, (64, DIN), "wxpT1", row0=128)
        wzT0 = wload(wzT, (128, DIN), "wzT0")
        wzT1 = wload(wzT, (64, DIN), "wzT1", row0=128)
        wxcT0 = wload(wxcT, (128, DIN), "wxcT0")
        wxcT1 = wload(wxcT, (64, DIN), "wxcT1", row0=128)
        wconv_s = wload(wconv, (128, 9 * NDT * 128), "wconv")
        convb_s = wload(convb, (128, NDT), "convb", dt=F32)
        xproj_s = wload(xprojT, (128, NDT * 128), "xproj")
        dtw_s = wload(dtwT, (R, DIN), "dtw")
        dtb_s = wload(dtbias, (128, NDT), "dtb", dt=F32)
        A_s = wload(Amat, (128, NDT * N), "Amat", dt=F32)
        D_s = wload(Dvec, (128, NDT), "Dvec", dt=F32)
        ones_s = wload(onesm, (128, 128), "ones")
        g_s = wload(gamma, (128, NDT), "gamma", dt=F32)
        bta_s = wload(beta, (128, NDT), "beta", dt=F32)
        wout_s = wload(outprojT, (128, NDT * DM), "wout")
        eps_s = wload(epsc, (128, 1), "epsc", dt=F32)

        # ---- residents
        xp_pad = [rpool.tile([128, PAD], BF16, tag=f"xp_pad{i}", name=f"xp_pad{i}")
                  for i in range(NDT)]
        delta = [rpool.tile([128, L], BF16, tag=f"delta{i}", name=f"delta{i}")
                 for i in range(NDT)]
        uu = [rpool.tile([128, L], BF16, tag=f"u{i}", name=f"u{i}")
              for i in range(NDT)]
        yac = [rpool.tile([128, L], F32, tag=f"yac{i}", name=f"yac{i}")
               for i in range(NDT)]
        xcs = [rpool.tile([128, L], BF16, tag=f"xcs{i}", name=f"xcs{i}")
               for i in range(NDT)]

        xdblB = rpool.tile([N, L], BF16, tag="xdblB", name="xdblB")
        xdblC = rpool.tile([N, L], BF16, tag="xdblC", name="xdblC")
        xdbl_dt = rpool.tile([R, L], BF16, tag="xdbl_dt", name="xdbl_dt")
        # B/C broadcast tiles, bf16, quad-grouped [128, QN, L]
        Bq = [bcpool.tile([128, QN, L], BF16, tag=f"Bq{q}", name=f"Bq{q}")
              for q in range(NQ)]
        Cq = [bcpool.tile([128, QN, L], BF16, tag=f"Cq{q}", name=f"Cq{q}")
              for q in range(NQ)]

        for i in range(NDT):
            nc.vector.memset(xp_pad[i][:], 0.0)

        # ---- input loads (transient slots)
        xT0 = tmp(tag="tmp")
        nc.sync.dma_start(xT0[:], xT[0:128, :])
        xT1 = tmp((64, L), tag="tmp")
        nc.sync.dma_start(xT1[:], xT[128:192, :])
        xcT0 = tmp(tag="tmp")
        nc.sync.dma_start(xcT0[:], xcT[0:128, :])
        xcT1 = tmp((64, L), tag="tmp")
        nc.sync.dma_start(xcT1[:], xcT[128:192, :])

        def proj_mm(w0, w1, r0, r1, i, half):
            ps = ps_bc.tile([128, 512], F32, tag="pm", name="pm")
            cs = slice(i * 128, (i + 1) * 128)
            hs = slice(half * 512, (half + 1) * 512)
            nc.tensor.matmul(ps[:], w0[:, cs], r0[:, hs], start=True, stop=False)
            nc.tensor.matmul(ps[:], w1[:, cs], r1[:, hs], start=False, stop=True)
            return ps

        # xc projection (feeds x_dbl -> B/C/delta: done first)
        for i in range(NDT):
            for half in range(2):
                ps = proj_mm(wxcT0, wxcT1, xcT0, xcT1, i, half)
                nc.scalar.copy(xcs[i][:, half * 512:(half + 1) * 512], ps[:])

        # ---- x_dbl = W_k @ xsc   [44, L] scattered at rows 0:R / 32:48 / 64:80
        for half in range(2):
            psx = ps_bc.tile([128, 512], F32, tag="pm", name="px")
            hs = slice(half * 512, (half + 1) * 512)
            for i in range(NDT):
                nc.tensor.matmul(psx[:], xproj_s[:, i * 128:(i + 1) * 128],
                                 xcs[i][:, hs], start=(i == 0), stop=(i == NDT - 1))
            nc.scalar.copy(xdbl_dt[:, hs], psx[0:R, :])
            nc.scalar.copy(xdblB[:, hs], psx[32:32 + N, :])
            nc.scalar.copy(xdblC[:, hs], psx[64:64 + N, :])

        # ---- B/C broadcast to 128 partitions via DRAM-bounce stride-0 DMAs.
        # Emitted per-quad, interleaved with the dtile-0 scan (see below), so
        # only quad 0 gates the first scan op.
        bcB = dram.tile([N, L], BF16, tag="bcB", name="bcB")
        bcC = dram.tile([N, L], BF16, tag="bcC", name="bcC")
        nc.sync.dma_start(bcB[:], xdblB[:])
        nc.sync.dma_start(bcC[:], xdblC[:])

        def emit_bc(q):
            for j in range(QN):
                n = q * QN + j
                s_ap, d_ap = bass.broadcast_tensor_aps(bcB[n:n + 1, :],
                                                       Bq[q][:, j, :])
                nc.sync.dma_start(d_ap, s_ap)
                s_ap, d_ap = bass.broadcast_tensor_aps(bcC[n:n + 1, :],
                                                       Cq[q][:, j, :])
                nc.sync.dma_start(d_ap, s_ap)

        # ---- per-dtile front-end: xp proj -> conv+silu -> dts -> delta/u/yac
        xh = [xp_pad[i][:, 0:L] for i in range(NDT)]

        def emit_front(i):
            for half in range(2):
                ps = proj_mm(wxpT0, wxpT1, xT0, xT1, i, half)
                dst = xp_pad[i][:].rearrange("p (h w) -> p h w", h=34)
                h0 = 1 + 16 * half
                nc.scalar.copy(dst[:, h0:h0 + 16, 1:33],
                               ps[:].rearrange("p (h w) -> p h w", h=16))
            psc = ps_bc.tile([128, 1024], F32, tag="pn", name="psconv")
            pad3 = xp_pad[i][:].rearrange("p (h w) -> p h w", h=34)
            for j in range(9):
                oh, ow = j // 3, j % 3
                wsl = wconv_s[:, (j * NDT + i) * 128:(j * NDT + i) * 128 + 128]
                for half in range(2):
                    h0 = oh + 16 * half
                    win = pad3[:, h0:h0 + 16, ow:ow + 32]
                    nc.tensor.matmul(psc[:, half * 512:(half + 1) * 512],
                                     wsl, win, start=(j == 0), stop=(j == 8))
            nc.scalar.activation(xh[i], psc[:], AF.Silu,
                                 bias=convb_s[:, i:i + 1], scale=1.0)
            psd = ps_bc.tile([128, 1024], F32, tag="pn", name="psd")
            for half in range(2):
                hs = slice(half * 512, (half + 1) * 512)
                nc.tensor.matmul(psd[:, hs], dtw_s[:, i * 128:(i + 1) * 128],
                                 xdbl_dt[:, hs], start=True, stop=True)
            et = tmp(tag="pa")
            nc.scalar.activation(et[:], psd[:], AF.Exp,
                                 bias=dtb_s[:, i:i + 1], scale=1.0)
            nc.scalar.activation(delta[i][:], et[:], AF.Ln, bias=1.0)
            nc.vector.tensor_tensor(uu[i][:], delta[i][:], xh[i], OP.mult)
            nc.vector.tensor_scalar_mul(yac[i][:], xh[i], D_s[:, i:i + 1])

        # ---- scan: dtile outer (per-dtile AllGather overlaps next dtile)
        ybounce = [dram.tile([128, L], BF16, tag=f"ybounce{i}", name=f"ybounce{i}")
                   for i in range(NDT)]
        ygather = [dram.tile([K * 128, L], BF16, tag=f"ygather{i}", name=f"ygather{i}")
                   for i in range(NDT)]
        emit_front(0)
        for i in range(NDT):
            for q in range(NQ):
                if i == 0:
                    emit_bc(q)
                a_q = apool.tile([128, QN, L], BF16, tag="a_q", name=f"a{i}{q}")
                for j in range(QN):
                    n = q * QN + j
                    nc.scalar.activation(a_q[:, j, :], delta[i][:], AF.Exp,
                                         scale=A_s[:, i * N + n:i * N + n + 1])
                nc.vector.memset(a_q[:, :, 0:1], 0.0)

                b_q = hpool.tile([128, QN, L], BF16, tag="b_q", name=f"b{i}{q}")
                a0, a1 = bass.broadcast_tensor_aps(uu[i][:, None, :], Bq[q][:])
                nc.vector.tensor_tensor(b_q[:], a0, a1, OP.mult)

                h_q = hpool.tile([128, QN, L], BF16, tag="h_q", name=f"h{i}{q}")
                nc.vector.tensor_tensor_scan(
                    h_q[:].rearrange("p n t -> p (n t)"),
                    a_q[:].rearrange("p n t -> p (n t)"),
                    b_q[:].rearrange("p n t -> p (n t)"),
                    0.0, OP.mult, OP.add)

                # hc into b_q (dead), pair-tree into h_q (dead), acc into yac
                nc.vector.tensor_tensor(b_q[:], h_q[:], Cq[q][:], OP.mult)
                nc.vector.tensor_tensor(h_q[:, 0:2, :], b_q[:, 0:2, :],
                                        b_q[:, 2:4, :], OP.add)
                nc.vector.tensor_tensor(h_q[:, 2, :], h_q[:, 0, :],
                                        h_q[:, 1, :], OP.add)
                nc.vector.tensor_tensor(yac[i][:], yac[i][:], h_q[:, 2, :], OP.add)

            ybc = tmp(tag="sg")
            nc.vector.tensor_copy(ybc[:], yac[i][:])
            nc.sync.dma_start(ybounce[i][:], ybc[:])
            nc.gpsimd.collective_compute(
                "AllGather", OP.bypass,
                replica_groups=[[0, 1, 2, 3], [4, 5, 6, 7]],
                ins=[ybounce[i][:].opt()], outs=[ygather[i][:].opt()])
            if i + 1 < NDT:
                emit_front(i + 1)

        # ---- merge across directions (unpermute each slot, add)
        yc = []
        for i in range(NDT):
            sl = [tmp(tag="tmp") for _ in range(K)]
            for k in range(K):
                nc.sync.dma_start(sl[k][:], ygather[i][k * 128:(k + 1) * 128, :])
            r1 = sl[1][:].rearrange("p (w h) -> p h w", w=32)
            r2 = sl[2][:, ::-1]
            r3 = sl[3][:, ::-1].rearrange("p (w h) -> p h w", w=32)
            t01 = spool.tile([128, L], BF16, tag="sc1", name="t01")
            nc.vector.tensor_tensor(t01[:].rearrange("p (h w) -> p h w", h=32),
                                    sl[0][:].rearrange("p (h w) -> p h w", h=32),
                                    r1, OP.add)
            t23 = spool.tile([128, L], BF16, tag="sc2", name="t23")
            nc.vector.tensor_tensor(t23[:].rearrange("p (h w) -> p h w", h=32),
                                    r2.rearrange("p (h w) -> p h w", h=32),
                                    r3, OP.add)
            yci = rpool.tile([128, L], BF16, tag=f"xcs{i}", name=f"yc{i}")
            nc.vector.tensor_tensor(yci[:], t01[:], t23[:], OP.add)
            yc.append(yci)

        # ---- z projection + silu
        xTc0 = spool.tile([128, L], BF16, tag="xtc0", name="xTc0")
        nc.sync.dma_start(xTc0[:], xTc[0:128, :])
        xTc1 = spool.tile([64, L], BF16, tag="xtc1", name="xTc1")
        nc.sync.dma_start(xTc1[:], xTc[128:192, :])
        # zsil reuses delta's buffers (delta is dead once dtile i's scan a_t
        # activations have been issued)
        zsil = []
        for i in range(NDT):
            zsi = rpool.tile([128, L], BF16, tag=f"delta{i}", name=f"zsil{i}")
            for half in range(2):
                ps = proj_mm(wzT0, wzT1, xTc0, xTc1, i, half)
                nc.scalar.activation(zsi[:, half * 512:(half + 1) * 512],
                                     ps[:], AF.Silu)
            zsil.append(zsi)

        # ---- LayerNorm stats (ones-matmul partition reduction, bf16 in, fp32 acc)
        ysq = []
        for i in range(NDT):
            sq = rpool.tile([128, L], BF16, tag=f"xp_pad{i}", name=f"ysq{i}")
            nc.scalar.activation(sq[:], yc[i][:], AF.Square)
            ysq.append(sq)
        psmu = ps_bc.tile([128, 1024], F32, tag="pn", name="psmu")
        psms = ps_bc.tile([128, 1024], F32, tag="pn", name="psms")
        for half in range(2):
            hs = slice(half * 512, (half + 1) * 512)
            for i in range(NDT):
                nc.tensor.matmul(psmu[:, hs], ones_s[:], yc[i][:, hs],
                                 start=(i == 0), stop=(i == NDT - 1))
            for i in range(NDT):
                nc.tensor.matmul(psms[:, hs], ones_s[:], ysq[i][:, hs],
                                 start=(i == 0), stop=(i == NDT - 1))
        mu_sb = spool.tile([128, L], F32, tag="mu", name="mu")
        nc.scalar.mul(mu_sb[:], psmu[:], 1.0 / DIN)
        musq = spool.tile([128, L], F32, tag="musq", name="musq")
        nc.vector.tensor_tensor(musq[:], mu_sb[:], mu_sb[:], OP.mult)
        ms_sb = spool.tile([128, L], F32, tag="vart", name="ms")
        nc.scalar.mul(ms_sb[:], psms[:], 1.0 / DIN)
        vart = spool.tile([128, L], F32, tag="d1", name="vart")
        nc.vector.tensor_tensor(vart[:], ms_sb[:], musq[:], OP.subtract)
        lnv = spool.tile([128, L], F32, tag="musq", name="lnv")
        nc.scalar.activation(lnv[:], vart[:], AF.Ln, bias=eps_s[:, 0:1])
        inv = spool.tile([128, L], F32, tag="vart", name="inv")
        nc.scalar.activation(inv[:], lnv[:], AF.Exp, scale=-0.5)

        # ---- normalize + gate + out projection
        yg = []
        for i in range(NDT):
            d1 = spool.tile([128, L], F32, tag="d1", name=f"d1_{i}")
            nc.vector.tensor_tensor(d1[:], yc[i][:], mu_sb[:], OP.subtract)
            d2 = spool.tile([128, L], F32, tag="d2", name=f"d2_{i}")
            nc.vector.tensor_tensor(d2[:], d1[:], inv[:], OP.mult)
            d3 = spool.tile([128, L], BF16, tag="sc1", name=f"d3_{i}")
            nc.scalar.activation(d3[:], d2[:], AF.Identity,
                                 bias=bta_s[:, i:i + 1], scale=g_s[:, i:i + 1])
            ygi = rpool.tile([128, L], BF16, tag=f"u{i}", name=f"yg{i}")
            nc.vector.tensor_tensor(ygi[:], d3[:], zsil[i][:], OP.mult)
            yg.append(ygi)

        for c in range(8):
            pso = ps_bc.tile([128, DM], F32, tag="pm", name="pso")
            for i in range(NDT):
                nc.tensor.matmul(pso[:], yg[i][:, c * 128:(c + 1) * 128],
                                 wout_s[:, i * DM:(i + 1) * DM],
                                 start=(i == 0), stop=(i == NDT - 1))
            ob = obpool.tile([128, DM], F32, tag="ob", name="ob")
            nc.scalar.copy(ob[:], pso[:])
            nc.sync.dma_start(out_d[c * 128:(c + 1) * 128, :], ob[:])

    nc.compile()
    return nc


def _prep_maps(inputs):
    x = np.asarray(inputs["x"], np.float32)
    x_cross = np.asarray(inputs["x_cross"], np.float32)
    in_proj_w = np.asarray(inputs["in_proj_w"], np.float32)
    in_proj_cross_w = np.asarray(inputs["in_proj_cross_w"], np.float32)
    conv_w = np.asarray(inputs["conv_w"], np.float32)
    conv_b = np.asarray(inputs["conv_b"], np.float32)
    x_proj_weight = np.asarray(inputs["x_proj_weight"], np.float32)
    dt_projs_weight = np.asarray(inputs["dt_projs_weight"], np.float32)
    dt_projs_bias = np.asarray(inputs["dt_projs_bias"], np.float32)
    A_logs = np.asarray(inputs["A_logs"], np.float32)
    Ds = np.asarray(inputs["Ds"], np.float32)
    out_norm_g = np.asarray(inputs["out_norm_g"], np.float32)
    out_norm_b = np.asarray(inputs["out_norm_b"], np.float32)
    out_proj_w = np.asarray(inputs["out_proj_w"], np.float32)

    W_xp = in_proj_w[:DIN]
    W_z = in_proj_w[DIN:2 * DIN]
    A_full = (-np.exp(A_logs)).reshape(K, DIN, N)
    Ds_k = Ds.reshape(K, DIN)

    def fold3(v):  # [384] -> [128, 3]
        return np.ascontiguousarray(v.reshape(NDT, 128).T)

    common = {
        "wxpT": np.ascontiguousarray(W_xp.T).astype(BF),
        "wzT": np.ascontiguousarray(W_z.T).astype(BF),
        "wxcT": np.ascontiguousarray(in_proj_cross_w.T).astype(BF),
        "convb": fold3(conv_b),
        "onesm": np.full((128, 128), 1.0, BF),
        "gamma": fold3(out_norm_g),
        "beta": fold3(out_norm_b),
        "outprojT": np.ascontiguousarray(
            out_proj_w.T.reshape(NDT, 128, DM).transpose(1, 0, 2)
            .reshape(128, NDT * DM)).astype(BF),
        "epsc": np.full((128, 1), 1e-5, np.float32),
    }


    in_maps = []
    for c in range(NCORES):
        b, k = c // 4, c % 4
        p = _perm(k)
        xb = x[b].reshape(L, DM)
        xcb = x_cross[b].reshape(L, DM)
        w = conv_w[:, 0]  # [384, 3, 3]
        if k == 0:
            wk = w
        elif k == 1:
            wk = w.transpose(0, 2, 1)
        elif k == 2:
            wk = w[:, ::-1, ::-1]
        else:
            wk = w.transpose(0, 2, 1)[:, ::-1, ::-1]
        wconv = np.zeros((128, 9 * NDT * 128), np.float32)
        for j in range(9):
            for i in range(NDT):
                m = j * NDT + i
                dgv = np.ascontiguousarray(wk[i * 128:(i + 1) * 128, j // 3, j % 3])
                wconv[:, m * 128:m * 128 + 128] = np.diag(dgv)
        xp_w = x_proj_weight[k]  # [44, 384]
        xp_scat = np.zeros((DIN, 128), np.float32)   # lhsT cols = out partition
        xp_scat[:, 0:R] = xp_w[0:R].T
        xp_scat[:, 32:32 + N] = xp_w[R:R + N].T
        xp_scat[:, 64:64 + N] = xp_w[R + N:R + 2 * N].T
        xproj = np.ascontiguousarray(
            xp_scat.reshape(NDT, 128, 128).transpose(1, 0, 2).reshape(128, NDT * 128))
        Am = np.ascontiguousarray(
            A_full[k].reshape(NDT, 128, N).transpose(1, 0, 2).reshape(128, NDT * N))
        m = dict(common)
        m.update({
            "xT": np.ascontiguousarray(xb[p].T).astype(BF),
            "xcT": np.ascontiguousarray(xcb[p].T).astype(BF),
            "xTc": np.ascontiguousarray(xb.T).astype(BF),
            "wconv": wconv.astype(BF),
            "xprojT": xproj.astype(BF),
            "dtwT": np.ascontiguousarray(dt_projs_weight[k].T).astype(BF),
            "dtbias": fold3(dt_projs_bias[k]),
            "Amat": Am,
            "Dvec": fold3(Ds_k[k]),
        })
        in_maps.append(m)
    return in_maps


def kernel(**inputs):
    if "nc" not in _cache:
        _cache["nc"] = _build_nc()
    nc = _cache["nc"]
    in_maps = _prep_maps(inputs)
    res = run_bass_kernel_spmd(nc, in_maps, core_ids=list(range(NCORES)))
    out = np.zeros((B_, L, DM), np.float32)
    out[0] = res.results[0]["out"]
    out[1] = res.results[4]["out"]
    return out.reshape(B_, HH, WW, DM)


# revision 28
# speedup vs baseline: 2.0155x; 1.0466x over previous
"""CSS2D (cross selective-scan 2D) Trainium2 kernel.

Sharding: 8 cores = batch(2) x scan-direction(4). Each core runs the full
pipeline for its (b, k) in the direction's own time order; direction
permutations are applied host-side to the inputs (and to the depthwise-conv
taps, which commute with grid transpose/reversal), so all 8 cores execute one
uniform SPMD program. The 4-direction merge is a per-dtile bf16 AllGather
within each b-group (overlapped with the scan of the next dtile) followed by
on-chip unpermute-and-add, LayerNorm, gating and the output projection
(computed redundantly per group; core 4b's output is used).

All matmuls run in bf16 (PE fp32 is 4 cycles/row vs 1 for bf16); the scan
elementwise chain runs in bf16 (DVE tensor_tensor 2x mode, quad-grouped
states) with the selective-scan internal state in fp32. Weights ship as three
packed blobs (one DMA each); B/C are broadcast to all 128 partitions with
stride-0 DRAM-bounce DMAs, one per (quad, B/C).
"""
import numpy as np
import ml_dtypes
from contextlib import ExitStack

import concourse.bacc as bacc
import concourse.bass as bass
import concourse.mybir as mybir
import concourse.tile as tile
from concourse.bass_utils import run_bass_kernel_spmd

F32 = mybir.dt.float32
BF16 = mybir.dt.bfloat16
AF = mybir.ActivationFunctionType
OP = mybir.AluOpType

B_, HH, WW = 2, 32, 32
L = HH * WW                    # 1024
DM, DIN, N, R, K = 192, 384, 16, 12, 4
NDT = DIN // 128               # 3 d-tiles
QN = 4                         # states per scan quad
NQ = N // QN                   # 4 quads
NCORES = 8
PAD = 34 * 34                  # padded conv plane
BF = ml_dtypes.bfloat16

# blob A (bf16, [128, .]) column offsets
O_WXP, O_WZ, O_WXC = 0, 384, 768
O_CONV = 1152                  # 9*3*128 = 3456
O_XPROJ = 4608
O_WOUT = 4992                  # 3*192 = 576
O_DTW = 5568                   # rows 0:12, 3*128
O_ONES = 5952
ABLOB = 6080
# blob F (f32, [128, .]) column offsets
F_CONVB, F_DTB, F_A, F_D, F_G, F_B, F_EPS = 0, 3, 6, 54, 57, 60, 63
FBLOB = 64

_cache = {}


def _perm(k):
    t = np.arange(L)
    if k == 0:
        return t
    if k == 1:
        return (t % 32) * 32 + t // 32
    if k == 2:
        return 1023 - t
    return _perm(1)[1023 - t]


def _build_nc():
    nc = bacc.Bacc(None, target_bir_lowering=False)

    wblobA = nc.declare_dram_parameter("wblobA", [128, ABLOB], BF16, isOutput=False)
    wblobB = nc.declare_dram_parameter("wblobB", [64, 1152], BF16, isOutput=False)
    wblobF = nc.declare_dram_parameter("wblobF", [128, FBLOB], F32, isOutput=False)
    xin = nc.declare_dram_parameter("xin", [192, 2 * L], BF16, isOutput=False)
    xTc = nc.declare_dram_parameter("xTc", [192, L], BF16, isOutput=False)
    out_d = nc.declare_dram_parameter("out", [L, DM], F32, isOutput=True)

    with ExitStack() as ctx:
        tc = ctx.enter_context(tile.TileContext(nc))
        wpool = ctx.enter_context(tc.tile_pool(name="w", bufs=1))
        rpool = ctx.enter_context(tc.tile_pool(name="r", bufs=1))
        tpool = ctx.enter_context(tc.tile_pool(name="t", bufs=2))
        iopool = ctx.enter_context(tc.tile_pool(name="io", bufs=4))
        bcpool = ctx.enter_context(tc.tile_pool(name="bcp", bufs=1))
        apool = ctx.enter_context(tc.tile_pool(name="a", bufs=2))
        hpool = ctx.enter_context(tc.tile_pool(name="h", bufs=1))
        spool = ctx.enter_context(tc.tile_pool(name="s", bufs=1))
        obpool = ctx.enter_context(tc.tile_pool(name="obp", bufs=2))
        ps_bc = ctx.enter_context(tc.tile_pool(name="psc", bufs=2, space="PSUM"))
        dram = ctx.enter_context(tc.tile_pool(name="dram", bufs=1, space="DRAM"))

        def tmp(shape=(128, L), tag="tmp", dt=BF16):
            pool = iopool if tag == "tmp" else tpool
            return pool.tile(list(shape), dt, tag=tag, name=tag)

        # ---- weight blobs (one DMA each)
        wA = wpool.tile([128, ABLOB], BF16, tag="wA", name="wA")
        nc.sync.dma_start(wA[:], wblobA[:, :])
        wB = wpool.tile([64, 1152], BF16, tag="wB", name="wB")
        nc.sync.dma_start(wB[:], wblobB[:, :])
        wF = wpool.tile([128, FBLOB], F32, tag="wF", name="wF")
        nc.sync.dma_start(wF[:], wblobF[:, :])

        # ---- residents
        xp_pad = [rpool.tile([128, PAD], BF16, tag=f"xp_pad{i}", name=f"xp_pad{i}")
                  for i in range(NDT)]
        delta = [rpool.tile([128, L], BF16, tag=f"delta{i}", name=f"delta{i}")
                 for i in range(NDT)]
        uu = [rpool.tile([128, L], BF16, tag=f"u{i}", name=f"u{i}")
              for i in range(NDT)]
        yac = [rpool.tile([128, L], BF16, tag=f"yac{i}", name=f"yac{i}")
               for i in range(NDT)]
        xcs = [rpool.tile([128, L], BF16, tag=f"xcs{i}", name=f"xcs{i}")
               for i in range(NDT)]
        xdblB = rpool.tile([N, L], BF16, tag="xdblB", name="xdblB")
        xdblC = rpool.tile([N, L], BF16, tag="xdblC", name="xdblC")
        xdbl_dt = rpool.tile([R, L], BF16, tag="xdbl_dt", name="xdbl_dt")
        # B/C broadcast tiles, bf16, quad-grouped [128, QN, L]
        Bq = [bcpool.tile([128, QN, L], BF16, tag=f"Bq{q}", name=f"Bq{q}")
              for q in range(NQ)]
        Cq = [bcpool.tile([128, QN, L], BF16, tag=f"Cq{q}", name=f"Cq{q}")
              for q in range(NQ)]

        for i in range(NDT):
            nc.vector.memset(xp_pad[i][:], 0.0)

        # ---- input loads (xT/xcT packed in one param)
        xin0 = rpool.tile([128, 2, L], BF16, tag="xin0", name="xin0")
        nc.sync.dma_start(xin0[:], xin[0:128, :].rearrange("p (s t) -> p s t", s=2))
        xin1 = rpool.tile([64, 2, L], BF16, tag="xin1", name="xin1")
        nc.sync.dma_start(xin1[:], xin[128:192, :].rearrange("p (s t) -> p s t", s=2))

        _WB_BASE = {O_WXP: 0, O_WZ: 384, O_WXC: 768}

        def proj_mm(wc0, rseg, i, half):
            ps = ps_bc.tile([128, 512], F32, tag="pm", name="pm")
            c0 = wc0 + i * 128
            b0 = _WB_BASE[wc0] + i * 128
            hs = slice(half * 512, (half + 1) * 512)
            nc.tensor.matmul(ps[:], wA[:, c0:c0 + 128], xin0[:, rseg, hs],
                             start=True, stop=False)
            nc.tensor.matmul(ps[:], wB[:, b0:b0 + 128],
                             xin1[:, rseg, hs], start=False, stop=True)
            return ps

        # xc projection (feeds x_dbl -> B/C/delta: done first)
        for i in range(NDT):
            for half in range(2):
                ps = proj_mm(O_WXC, 1, i, half)
                nc.scalar.copy(xcs[i][:, half * 512:(half + 1) * 512], ps[:])

        # ---- x_dbl = W_k @ xsc   [44, L] scattered at rows 0:R / 32:48 / 64:80
        for half in range(2):
            psx = ps_bc.tile([128, 512], F32, tag="pm", name="px")
            hs = slice(half * 512, (half + 1) * 512)
            for i in range(NDT):
                nc.tensor.matmul(psx[:], wA[:, O_XPROJ + i * 128:O_XPROJ + (i + 1) * 128],
                                 xcs[i][:, hs], start=(i == 0), stop=(i == NDT - 1))
            nc.scalar.copy(xdbl_dt[:, hs], psx[0:R, :])
            nc.scalar.copy(xdblB[:, hs], psx[32:32 + N, :])
            nc.scalar.copy(xdblC[:, hs], psx[64:64 + N, :])

        # ---- B/C broadcast to 128 partitions via DRAM-bounce stride-0 DMAs,
        # one DMA per (quad, B/C). Interleaved with the dtile-0 scan below.
        bcB = dram.tile([N, L], BF16, tag="bcB", name="bcB")
        bcC = dram.tile([N, L], BF16, tag="bcC", name="bcC")
        nc.sync.dma_start(bcB[:], xdblB[:])
        nc.sync.dma_start(bcC[:], xdblC[:])

        def emit_bc(q):
            for src, dst in ((bcB, Bq[q]), (bcC, Cq[q])):
                s_ap, d_ap = bass.broadcast_tensor_aps(
                    src[None, q * QN:(q + 1) * QN, :], dst[:])
                nc.sync.dma_start(d_ap, s_ap)

        # ---- per-dtile front-end: xp proj -> conv+silu -> dts -> delta/u/yac
        xh = [xp_pad[i][:, 0:L] for i in range(NDT)]

        def emit_front(i):
            for half in range(2):
                ps = proj_mm(O_WXP, 0, i, half)
                dst = xp_pad[i][:].rearrange("p (h w) -> p h w", h=34)
                h0 = 1 + 16 * half
                nc.scalar.copy(dst[:, h0:h0 + 16, 1:33],
                               ps[:].rearrange("p (h w) -> p h w", h=16))
            psc = ps_bc.tile([128, 1024], F32, tag="pn", name="psconv")
            pad3 = xp_pad[i][:].rearrange("p (h w) -> p h w", h=34)
            for j in range(9):
                oh, ow = j // 3, j % 3
                c0 = O_CONV + (j * NDT + i) * 128
                wsl = wA[:, c0:c0 + 128]
                for half in range(2):
                    h0 = oh + 16 * half
                    win = pad3[:, h0:h0 + 16, ow:ow + 32]
                    nc.tensor.matmul(psc[:, half * 512:(half + 1) * 512],
                                     wsl, win, start=(j == 0), stop=(j == 8))
            nc.scalar.activation(xh[i], psc[:], AF.Silu,
                                 bias=wF[:, F_CONVB + i:F_CONVB + i + 1], scale=1.0)
            psd = ps_bc.tile([128, 1024], F32, tag="pn", name="psd")
            for half in range(2):
                hs = slice(half * 512, (half + 1) * 512)
                nc.tensor.matmul(psd[:, hs],
                                 wA[0:R, O_DTW + i * 128:O_DTW + (i + 1) * 128],
                                 xdbl_dt[:, hs], start=True, stop=True)
            et = tmp(tag="pa")
            nc.scalar.activation(et[:], psd[:], AF.Exp,
                                 bias=wF[:, F_DTB + i:F_DTB + i + 1], scale=1.0)
            nc.scalar.activation(delta[i][:], et[:], AF.Ln, bias=1.0)
            nc.vector.tensor_tensor(uu[i][:], delta[i][:], xh[i], OP.mult)
            nc.vector.tensor_scalar_mul(yac[i][:], xh[i], wF[:, F_D + i:F_D + i + 1])

        # ---- scan: dtile outer (per-dtile AllGather overlaps next dtile)
        ybounce = [dram.tile([128, L], BF16, tag=f"ybounce{i}", name=f"ybounce{i}")
                   for i in range(NDT)]
        ygather = [dram.tile([K * 128, L], BF16, tag=f"ygather{i}", name=f"ygather{i}")
                   for i in range(NDT)]
        emit_front(0)
        for i in range(NDT):
            for q in range(NQ):
                if i == 0:
                    emit_bc(q)
                a_q = apool.tile([128, QN, L], BF16, tag="a_q", name=f"a{i}{q}")
                for j in range(QN):
                    n = q * QN + j
                    nc.scalar.activation(a_q[:, j, :], delta[i][:], AF.Exp,
                                         scale=wF[:, F_A + i * N + n:F_A + i * N + n + 1])
                nc.vector.memset(a_q[:, :, 0:1], 0.0)

                b_q = hpool.tile([128, QN, L], BF16, tag="b_q", name=f"b{i}{q}")
                a0, a1 = bass.broadcast_tensor_aps(uu[i][:, None, :], Bq[q][:])
                nc.vector.tensor_tensor(b_q[:], a0, a1, OP.mult)

                h_q = hpool.tile([128, QN, L], BF16, tag="h_q", name=f"h{i}{q}")
                nc.vector.tensor_tensor_scan(
                    h_q[:].rearrange("p n t -> p (n t)"),
                    a_q[:].rearrange("p n t -> p (n t)"),
                    b_q[:].rearrange("p n t -> p (n t)"),
                    0.0, OP.mult, OP.add)

                # hc into b_q (dead), pair-tree into h_q (dead), acc into yac
                nc.vector.tensor_tensor(b_q[:], h_q[:], Cq[q][:], OP.mult)
                nc.vector.tensor_tensor(h_q[:, 0:2, :], b_q[:, 0:2, :],
                                        b_q[:, 2:4, :], OP.add)
                nc.vector.tensor_tensor(h_q[:, 2, :], h_q[:, 0, :],
                                        h_q[:, 1, :], OP.add)
                nc.vector.tensor_tensor(yac[i][:], yac[i][:], h_q[:, 2, :], OP.add)

            nc.sync.dma_start(ybounce[i][:], yac[i][:])
            nc.gpsimd.collective_compute(
                "AllGather", OP.bypass,
                replica_groups=[[0, 1, 2, 3], [4, 5, 6, 7]],
                ins=[ybounce[i][:].opt()], outs=[ygather[i][:].opt()])
            if i + 1 < NDT:
                emit_front(i + 1)

        # ---- merge across directions (unpermute each slot, add)
        yc = []
        for i in range(NDT):
            sl = [tmp(tag="tmp") for _ in range(K)]
            for k in range(K):
                nc.sync.dma_start(sl[k][:], ygather[i][k * 128:(k + 1) * 128, :])
            r1 = sl[1][:].rearrange("p (w h) -> p h w", w=32)
            r2 = sl[2][:, ::-1]
            r3 = sl[3][:, ::-1].rearrange("p (w h) -> p h w", w=32)
            t01 = spool.tile([128, L], BF16, tag="sc1", name="t01")
            nc.vector.tensor_tensor(t01[:].rearrange("p (h w) -> p h w", h=32),
                                    sl[0][:].rearrange("p (h w) -> p h w", h=32),
                                    r1, OP.add)
            t23 = spool.tile([128, L], BF16, tag="sc2", name="t23")
            nc.vector.tensor_tensor(t23[:].rearrange("p (h w) -> p h w", h=32),
                                    r2.rearrange("p (h w) -> p h w", h=32),
                                    r3, OP.add)
            yci = rpool.tile([128, L], BF16, tag=f"xcs{i}", name=f"yc{i}")
            nc.vector.tensor_tensor(yci[:], t01[:], t23[:], OP.add)
            yc.append(yci)

        # ---- z projection + silu (zsil reuses delta's buffers)
        xTc0 = spool.tile([128, 2, L // 2], BF16, tag="xtc0", name="xTc0")
        nc.sync.dma_start(xTc0[:], xTc[0:128, :].rearrange("p (s t) -> p s t", s=2))
        xTc1 = spool.tile([64, 2, L // 2], BF16, tag="xtc1", name="xTc1")
        nc.sync.dma_start(xTc1[:], xTc[128:192, :].rearrange("p (s t) -> p s t", s=2))
        zsil = []
        for i in range(NDT):
            zsi = rpool.tile([128, L], BF16, tag=f"delta{i}", name=f"zsil{i}")
            for half in range(2):
                ps = ps_bc.tile([128, 512], F32, tag="pm", name="pm")
                c0 = O_WZ + i * 128
                nc.tensor.matmul(ps[:], wA[:, c0:c0 + 128], xTc0[:, half, :],
                                 start=True, stop=False)
                nc.tensor.matmul(ps[:], wB[:, 384 + i * 128:384 + (i + 1) * 128],
                                 xTc1[:, half, :], start=False, stop=True)
                nc.scalar.activation(zsi[:, half * 512:(half + 1) * 512],
                                     ps[:], AF.Silu)
            zsil.append(zsi)

        # ---- LayerNorm stats (ones-matmul partition reduction, bf16 in, fp32 acc)
        ysq = []
        for i in range(NDT):
            sq = rpool.tile([128, L], BF16, tag=f"xp_pad{i}", name=f"ysq{i}")
            nc.scalar.activation(sq[:], yc[i][:], AF.Square)
            ysq.append(sq)
        psmu = ps_bc.tile([128, 1024], F32, tag="pn", name="psmu")
        psms = ps_bc.tile([128, 1024], F32, tag="pn", name="psms")
        for half in range(2):
            hs = slice(half * 512, (half + 1) * 512)
            for i in range(NDT):
                nc.tensor.matmul(psmu[:, hs], wA[:, O_ONES:O_ONES + 128],
                                 yc[i][:, hs], start=(i == 0), stop=(i == NDT - 1))
            for i in range(NDT):
                nc.tensor.matmul(psms[:, hs], wA[:, O_ONES:O_ONES + 128],
                                 ysq[i][:, hs], start=(i == 0), stop=(i == NDT - 1))
        mu_sb = spool.tile([128, L], F32, tag="mu", name="mu")
        nc.scalar.mul(mu_sb[:], psmu[:], 1.0 / DIN)
        musq = spool.tile([128, L], F32, tag="musq", name="musq")
        nc.scalar.activation(musq[:], mu_sb[:], AF.Square)
        ms_sb = spool.tile([128, L], F32, tag="vart", name="ms")
        nc.scalar.mul(ms_sb[:], psms[:], 1.0 / DIN)
        vart = spool.tile([128, L], F32, tag="d1", name="vart")
        nc.vector.tensor_tensor(vart[:], ms_sb[:], musq[:], OP.subtract)
        lnv = spool.tile([128, L], F32, tag="musq", name="lnv")
        nc.scalar.activation(lnv[:], vart[:], AF.Ln,
                             bias=wF[:, F_EPS:F_EPS + 1])
        inv = spool.tile([128, L], F32, tag="vart", name="inv")
        nc.scalar.activation(inv[:], lnv[:], AF.Exp, scale=-0.5)

        # ---- normalize + gate + out projection
        yg = []
        for i in range(NDT):
            d1 = spool.tile([128, L], F32, tag="d1", name=f"d1_{i}")
            nc.vector.tensor_tensor(d1[:], yc[i][:], mu_sb[:], OP.subtract)
            d2 = spool.tile([128, L], F32, tag="d2", name=f"d2_{i}")
            nc.vector.tensor_tensor(d2[:], d1[:], inv[:], OP.mult)
            d3 = spool.tile([128, L], BF16, tag="sc1", name=f"d3_{i}")
            nc.scalar.activation(d3[:], d2[:], AF.Identity,
                                 bias=wF[:, F_B + i:F_B + i + 1],
                                 scale=wF[:, F_G + i:F_G + i + 1])
            ygi = rpool.tile([128, L], BF16, tag=f"u{i}", name=f"yg{i}")
            nc.vector.tensor_tensor(ygi[:], d3[:], zsil[i][:], OP.mult)
            yg.append(ygi)

        for c in range(8):
            pso = ps_bc.tile([128, DM], F32, tag="pm", name="pso")
            for i in range(NDT):
                nc.tensor.matmul(pso[:], yg[i][:, c * 128:(c + 1) * 128],
                                 wA[:, O_WOUT + i * DM:O_WOUT + (i + 1) * DM],
                                 start=(i == 0), stop=(i == NDT - 1))
            ob = obpool.tile([128, DM], F32, tag="ob", name="ob")
            nc.scalar.copy(ob[:], pso[:])
            nc.sync.dma_start(out_d[c * 128:(c + 1) * 128, :], ob[:])

    nc.compile()
    return nc


def _prep_maps(inputs):
    x = np.asarray(inputs["x"], np.float32)
    x_cross = np.asarray(inputs["x_cross"], np.float32)
    in_proj_w = np.asarray(inputs["in_proj_w"], np.float32)
    in_proj_cross_w = np.asarray(inputs["in_proj_cross_w"], np.float32)
    conv_w = np.asarray(inputs["conv_w"], np.float32)
    conv_b = np.asarray(inputs["conv_b"], np.float32)
    x_proj_weight = np.asarray(inputs["x_proj_weight"], np.float32)
    dt_projs_weight = np.asarray(inputs["dt_projs_weight"], np.float32)
    dt_projs_bias = np.asarray(inputs["dt_projs_bias"], np.float32)
    A_logs = np.asarray(inputs["A_logs"], np.float32)
    Ds = np.asarray(inputs["Ds"], np.float32)
    out_norm_g = np.asarray(inputs["out_norm_g"], np.float32)
    out_norm_b = np.asarray(inputs["out_norm_b"], np.float32)
    out_proj_w = np.asarray(inputs["out_proj_w"], np.float32)

    W_xp = in_proj_w[:DIN]
    W_z = in_proj_w[DIN:2 * DIN]
    A_full = (-np.exp(A_logs)).reshape(K, DIN, N)
    Ds_k = Ds.reshape(K, DIN)

    def fold3(v):  # [384] -> [128, 3]
        return np.ascontiguousarray(v.reshape(NDT, 128).T)

    # blob A/B common pieces
    def wsplit(wT):  # [192, 384] -> rows 0:128 / 128:192
        return wT[0:128], wT[128:192]

    wxpT = np.ascontiguousarray(W_xp.T)
    wzT = np.ascontiguousarray(W_z.T)
    wxcT = np.ascontiguousarray(in_proj_cross_w.T)
    outT = np.ascontiguousarray(
        out_proj_w.T.reshape(NDT, 128, DM).transpose(1, 0, 2).reshape(128, NDT * DM))

    blobB = np.concatenate([wxpT[128:], wzT[128:], wxcT[128:]], axis=1).astype(BF)

    blobF = np.zeros((128, FBLOB), np.float32)
    blobF[:, F_CONVB:F_CONVB + 3] = fold3(conv_b)
    blobF[:, F_D:F_D + 3] = 0.0  # per-core below
    blobF[:, F_G:F_G + 3] = fold3(out_norm_g)
    blobF[:, F_B:F_B + 3] = fold3(out_norm_b)
    blobF[:, F_EPS] = 1e-5

    in_maps = []
    for c in range(NCORES):
        b, k = c // 4, c % 4
        p = _perm(k)
        xb = x[b].reshape(L, DM)
        xcb = x_cross[b].reshape(L, DM)
        w = conv_w[:, 0]  # [384, 3, 3]
        if k == 0:
            wk = w
        elif k == 1:
            wk = w.transpose(0, 2, 1)
        elif k == 2:
            wk = w[:, ::-1, ::-1]
        else:
            wk = w.transpose(0, 2, 1)[:, ::-1, ::-1]
        wconv = np.zeros((128, 9 * NDT * 128), np.float32)
        for j in range(9):
            for i in range(NDT):
                m = j * NDT + i
                dgv = np.ascontiguousarray(wk[i * 128:(i + 1) * 128, j // 3, j % 3])
                wconv[:, m * 128:m * 128 + 128] = np.diag(dgv)
        xp_w = x_proj_weight[k]  # [44, 384]
        xp_scat = np.zeros((DIN, 128), np.float32)   # lhsT cols = out partition
        xp_scat[:, 0:R] = xp_w[0:R].T
        xp_scat[:, 32:32 + N] = xp_w[R:R + N].T
        xp_scat[:, 64:64 + N] = xp_w[R + N:R + 2 * N].T
        xproj = np.ascontiguousarray(
            xp_scat.reshape(NDT, 128, 128).transpose(1, 0, 2).reshape(128, NDT * 128))
        dtw = np.zeros((128, NDT * 128), np.float32)
        dtw[0:R] = np.ascontiguousarray(dt_projs_weight[k].T)
        Am = np.ascontiguousarray(
            A_full[k].reshape(NDT, 128, N).transpose(1, 0, 2).reshape(128, NDT * N))

        blobA = np.zeros((128, ABLOB), np.float32)
        blobA[:, O_WXP:O_WXP + 384] = wxpT[0:128]
        blobA[:, O_WZ:O_WZ + 384] = wzT[0:128]
        blobA[:, O_WXC:O_WXC + 384] = wxcT[0:128]
        blobA[:, O_CONV:O_CONV + 3456] = wconv
        blobA[:, O_XPROJ:O_XPROJ + 384] = xproj
        blobA[:, O_WOUT:O_WOUT + 576] = outT
        blobA[:, O_DTW:O_DTW + 384] = dtw
        blobA[:, O_ONES:O_ONES + 128] = 1.0

        bF = blobF.copy()
        bF[:, F_DTB:F_DTB + 3] = fold3(dt_projs_bias[k])
        bF[:, F_A:F_A + NDT * N] = Am
        bF[:, F_D:F_D + 3] = fold3(Ds_k[k])

        xT = np.ascontiguousarray(xb[p].T)
        xcT = np.ascontiguousarray(xcb[p].T)
        m = {
            "wblobA": blobA.astype(BF),
            "wblobB": blobB,
            "wblobF": bF,
            "xin": np.concatenate([xT, xcT], axis=1).astype(BF),
            "xTc": np.ascontiguousarray(xb.T).astype(BF),
        }
        in_maps.append(m)
    return in_maps


def kernel(**inputs):
    if "nc" not in _cache:
        _cache["nc"] = _build_nc()
    nc = _cache["nc"]
    in_maps = _prep_maps(inputs)
    res = run_bass_kernel_spmd(nc, in_maps, core_ids=list(range(NCORES)))
    out = np.zeros((B_, L, DM), np.float32)
    out[0] = res.results[0]["out"]
    out[1] = res.results[4]["out"]
    return out.reshape(B_, HH, WW, DM)


# revision 32
# speedup vs baseline: 2.0215x; 1.0030x over previous
"""CSS2D (cross selective-scan 2D) Trainium2 kernel.

Sharding: 8 cores = batch(2) x scan-direction(4). Each core runs the full
pipeline for its (b, k) in the direction's own time order; direction
permutations are applied host-side to the inputs (and to the depthwise-conv
taps, which commute with grid transpose/reversal), so all 8 cores execute one
uniform SPMD program. The 4-direction merge is a per-dtile bf16 AllGather
within each b-group (overlapped with the scan of the next dtile) followed by
on-chip unpermute-and-add, LayerNorm, gating and the output projection
(computed redundantly per group; core 4b's output is used).

Key optimizations vs the straightforward mapping:
- all matmuls in bf16 (PE fp32 is 4 cycles/row vs 1 for bf16)
- the x_cross projection is folded host-side into the x_dbl / dt-projection
  weights (x_dbl = (W_k W_xc) @ x_cross), removing a full [384,192] GEMM
- scan elementwise chain in bf16 (DVE 2x mode, quad-grouped states); scan
  internal state fp32
- B/C broadcast to 128 partitions via stride-0 DRAM-bounce DMAs
- silu expressed through the exp/ln activation table (sigma(x) =
  exp(-ln(1+exp(-x)))) so the scalar engine never switches tables
- weights ship as three packed blobs (one DMA each)
"""
import numpy as np
import ml_dtypes
from contextlib import ExitStack

import concourse.bacc as bacc
import concourse.bass as bass
import concourse.mybir as mybir
import concourse.tile as tile
from concourse.bass_utils import run_bass_kernel_spmd

F32 = mybir.dt.float32
BF16 = mybir.dt.bfloat16
AF = mybir.ActivationFunctionType
OP = mybir.AluOpType

B_, HH, WW = 2, 32, 32
L = HH * WW                    # 1024
DM, DIN, N, R, K = 192, 384, 16, 12, 4
NDT = DIN // 128               # 3 d-tiles
QN = 4                         # states per scan quad
NQ = N // QN                   # 4 quads
NCORES = 8
PAD = 34 * 34                  # padded conv plane
BF = ml_dtypes.bfloat16

# blob A (bf16, [128, .]) column offsets
O_WXP, O_WZ, O_WDD = 0, 384, 768
O_CONV = 1152                  # 9*3*128 = 3456
O_XBC = 4608                   # 64 cols (B rows 0:16, C rows 32:48)
O_WOUT = 4672                  # 3*192 = 576
O_ONES = 5248
ABLOB = 5376
# blob B (bf16, [64, .]): wxpT1 @0, wzT1 @384, wddT1 @768, xbc1 @1152
BBLOB = 1216
# blob F (f32, [128, .]) column offsets
F_CONVB, F_CONVBN, F_DTB, F_A, F_D, F_G, F_B, F_EPS = 0, 3, 6, 9, 57, 60, 63, 66
FBLOB = 67

_cache = {}


def _perm(k):
    t = np.arange(L)
    if k == 0:
        return t
    if k == 1:
        return (t % 32) * 32 + t // 32
    if k == 2:
        return 1023 - t
    return _perm(1)[1023 - t]


def _build_nc():
    nc = bacc.Bacc(None, target_bir_lowering=False)

    wblobA = nc.declare_dram_parameter("wblobA", [128, ABLOB], BF16, isOutput=False)
    wblobB = nc.declare_dram_parameter("wblobB", [64, BBLOB], BF16, isOutput=False)
    wblobF = nc.declare_dram_parameter("wblobF", [128, FBLOB], F32, isOutput=False)
    xin = nc.declare_dram_parameter("xin", [192, 2 * L], BF16, isOutput=False)
    xTc = nc.declare_dram_parameter("xTc", [192, L], BF16, isOutput=False)
    out_d = nc.declare_dram_parameter("out", [L, DM], F32, isOutput=True)

    with ExitStack() as ctx:
        tc = ctx.enter_context(tile.TileContext(nc))
        wpool = ctx.enter_context(tc.tile_pool(name="w", bufs=1))
        rpool = ctx.enter_context(tc.tile_pool(name="r", bufs=1))
        tpool = ctx.enter_context(tc.tile_pool(name="t", bufs=2))
        iopool = ctx.enter_context(tc.tile_pool(name="io", bufs=4))
        bcpool = ctx.enter_context(tc.tile_pool(name="bcp", bufs=1))
        apool = ctx.enter_context(tc.tile_pool(name="a", bufs=2))
        hpool = ctx.enter_context(tc.tile_pool(name="h", bufs=1))
        spool = ctx.enter_context(tc.tile_pool(name="s", bufs=1))
        obpool = ctx.enter_context(tc.tile_pool(name="obp", bufs=2))
        ps_bc = ctx.enter_context(tc.tile_pool(name="psc", bufs=2, space="PSUM"))
        dram = ctx.enter_context(tc.tile_pool(name="dram", bufs=1, space="DRAM"))

        def tmp(shape=(128, L), tag="tmp", dt=BF16):
            pool = iopool if tag == "tmp" else tpool
            return pool.tile(list(shape), dt, tag=tag, name=tag)

        # ---- weight blobs (one DMA each)
        wA = wpool.tile([128, ABLOB], BF16, tag="wA", name="wA")
        nc.sync.dma_start(wA[:], wblobA[:, :])
        wB = wpool.tile([64, BBLOB], BF16, tag="wB", name="wB")
        nc.sync.dma_start(wB[:], wblobB[:, :])
        wF = wpool.tile([128, FBLOB], F32, tag="wF", name="wF")
        nc.sync.dma_start(wF[:], wblobF[:, :])

        # ---- residents
        xp_pad = [rpool.tile([128, PAD], BF16, tag=f"xp_pad{i}", name=f"xp_pad{i}")
                  for i in range(NDT)]
        delta = [rpool.tile([128, L], BF16, tag=f"delta{i}", name=f"delta{i}")
                 for i in range(NDT)]
        uu = [rpool.tile([128, L], BF16, tag=f"u{i}", name=f"u{i}")
              for i in range(NDT)]
        yac = [rpool.tile([128, L], BF16, tag=f"yac{i}", name=f"yac{i}")
               for i in range(NDT)]
        xdblB = rpool.tile([N, L], BF16, tag="xdblB", name="xdblB")
        xdblC = rpool.tile([N, L], BF16, tag="xdblC", name="xdblC")
        # B/C broadcast tiles, bf16, quad-grouped [128, QN, L]
        Bq = [bcpool.tile([128, QN, L], BF16, tag=f"Bq{q}", name=f"Bq{q}")
              for q in range(NQ)]
        Cq = [bcpool.tile([128, QN, L], BF16, tag=f"Cq{q}", name=f"Cq{q}")
              for q in range(NQ)]

        for i in range(NDT):
            nc.vector.memset(xp_pad[i][:], 0.0)

        # ---- input loads (xT/xcT packed in one param)
        xin0 = rpool.tile([128, 2, L], BF16, tag="xin0", name="xin0")
        nc.sync.dma_start(xin0[:], xin[0:128, :].rearrange("p (s t) -> p s t", s=2))
        xin1 = rpool.tile([64, 2, L], BF16, tag="xin1", name="xin1")
        nc.sync.dma_start(xin1[:], xin[128:192, :].rearrange("p (s t) -> p s t", s=2))

        _WB_BASE = {O_WXP: 0, O_WZ: 384, O_WDD: 768}

        def proj_mm(wc0, i, rhs0, rhs1):
            ps = ps_bc.tile([128, 512], F32, tag="pm", name="pm")
            c0 = wc0 + i * 128
            b0 = _WB_BASE[wc0] + i * 128
            nc.tensor.matmul(ps[:], wA[:, c0:c0 + 128], rhs0,
                             start=True, stop=False)
            nc.tensor.matmul(ps[:], wB[:, b0:b0 + 128], rhs1,
                             start=False, stop=True)
            return ps

        # ---- x_dbl = (W_k W_xc) @ x_cross^T : B rows 0:16, C rows 16:32
        for half in range(2):
            psx = ps_bc.tile([128, 512], F32, tag="pm", name="px")
            hs = slice(half * 512, (half + 1) * 512)
            nc.tensor.matmul(psx[0:64, :], wA[:, O_XBC:O_XBC + 64],
                             xin0[:, 1, hs], start=True, stop=False)
            nc.tensor.matmul(psx[0:64, :], wB[:, 1152:1152 + 64],
                             xin1[:, 1, hs], start=False, stop=True)
            nc.scalar.copy(xdblB[:, hs], psx[0:N, :])
            nc.scalar.copy(xdblC[:, hs], psx[32:32 + N, :])

        # ---- B/C broadcast to 128 partitions via DRAM-bounce stride-0 DMAs,
        # one DMA per (quad, B/C). Interleaved with the dtile-0 scan below.
        bcB = dram.tile([N, L], BF16, tag="bcB", name="bcB")
        bcC = dram.tile([N, L], BF16, tag="bcC", name="bcC")
        nc.sync.dma_start(bcB[:], xdblB[:])
        nc.sync.dma_start(bcC[:], xdblC[:])

        def emit_bc(q):
            for src, dst in ((bcB, Bq[q]), (bcC, Cq[q])):
                s_ap, d_ap = bass.broadcast_tensor_aps(
                    src[None, q * QN:(q + 1) * QN, :], dst[:])
                nc.sync.dma_start(d_ap, s_ap)

        def silu_sigmoid(src_ap, biasn):
            """sigma(src + bias) = exp(-ln(1 + exp(-(src+bias)))) on the
            exp/ln table. biasn is an AP holding the NEGATED bias (or 0.0)."""
            e1 = tmp(tag="sg")
            nc.scalar.activation(e1[:], src_ap, AF.Exp, scale=-1.0, bias=biasn)
            l1 = tmp(tag="sg")
            nc.scalar.activation(l1[:], e1[:], AF.Ln, bias=1.0)
            sg = tmp(tag="sg")
            nc.scalar.activation(sg[:], l1[:], AF.Exp, scale=-1.0)
            return sg

        # ---- per-dtile front-end: xp proj -> conv+silu -> dt -> delta/u/yac
        xh = [xp_pad[i][:, 0:L] for i in range(NDT)]

        def emit_front(i):
            for half in range(2):
                ps = proj_mm(O_WXP, i, xin0[:, 0, half * 512:(half + 1) * 512],
                             xin1[:, 0, half * 512:(half + 1) * 512])
                dst = xp_pad[i][:].rearrange("p (h w) -> p h w", h=34)
                h0 = 1 + 16 * half
                nc.scalar.copy(dst[:, h0:h0 + 16, 1:33],
                               ps[:].rearrange("p (h w) -> p h w", h=16))
            psc = ps_bc.tile([128, 1024], F32, tag="pn", name="psconv")
            pad3 = xp_pad[i][:].rearrange("p (h w) -> p h w", h=34)
            for j in range(9):
                oh, ow = j // 3, j % 3
                c0 = O_CONV + (j * NDT + i) * 128
                wsl = wA[:, c0:c0 + 128]
                for half in range(2):
                    h0 = oh + 16 * half
                    win = pad3[:, h0:h0 + 16, ow:ow + 32]
                    nc.tensor.matmul(psc[:, half * 512:(half + 1) * 512],
                                     wsl, win, start=(j == 0), stop=(j == 8))
            # conv out + bias in SBUF; silu via exp/ln table
            xpc = tmp(tag="pa")
            nc.scalar.activation(xpc[:], psc[:], AF.Identity,
                                 bias=wF[:, F_CONVB + i:F_CONVB + i + 1])
            sg = silu_sigmoid(psc[:], wF[:, F_CONVBN + i:F_CONVBN + i + 1])
            nc.vector.tensor_tensor(xh[i], xpc[:], sg[:], OP.mult)
            # delta = softplus((W_dd @ xc) + dtb) via exp/ln
            psd = ps_bc.tile([128, 1024], F32, tag="pn", name="psd")
            for half in range(2):
                ps = proj_mm(O_WDD, i, xin0[:, 1, half * 512:(half + 1) * 512],
                             xin1[:, 1, half * 512:(half + 1) * 512])
                nc.scalar.copy(psd[:, half * 512:(half + 1) * 512], ps[:])
            et = tmp(tag="pa")
            nc.scalar.activation(et[:], psd[:], AF.Exp,
                                 bias=wF[:, F_DTB + i:F_DTB + i + 1], scale=1.0)
            nc.scalar.activation(delta[i][:], et[:], AF.Ln, bias=1.0)
            nc.vector.tensor_tensor(uu[i][:], delta[i][:], xh[i], OP.mult)
            nc.vector.tensor_scalar_mul(yac[i][:], xh[i], wF[:, F_D + i:F_D + i + 1])

        # ---- scan: dtile outer (per-dtile AllGather overlaps next dtile)
        ybounce = [dram.tile([128, L], BF16, tag=f"ybounce{i}", name=f"ybounce{i}")
                   for i in range(NDT)]
        ygather = [dram.tile([K * 128, L], BF16, tag=f"ygather{i}", name=f"ygather{i}")
                   for i in range(NDT)]
        emit_front(0)
        for i in range(NDT):
            for q in range(NQ):
                if i == 0:
                    emit_bc(q)
                a_q = apool.tile([128, QN, L], BF16, tag="a_q", name=f"a{i}{q}")
                for j in range(QN):
                    n = q * QN + j
                    nc.scalar.activation(a_q[:, j, :], delta[i][:], AF.Exp,
                                         scale=wF[:, F_A + i * N + n:F_A + i * N + n + 1])
                nc.vector.memset(a_q[:, :, 0:1], 0.0)

                b_q = hpool.tile([128, QN, L], BF16, tag="b_q", name=f"b{i}{q}")
                a0, a1 = bass.broadcast_tensor_aps(uu[i][:, None, :], Bq[q][:])
                nc.vector.tensor_tensor(b_q[:], a0, a1, OP.mult)

                h_q = hpool.tile([128, QN, L], BF16, tag="h_q", name=f"h{i}{q}")
                nc.vector.tensor_tensor_scan(
                    h_q[:].rearrange("p n t -> p (n t)"),
                    a_q[:].rearrange("p n t -> p (n t)"),
                    b_q[:].rearrange("p n t -> p (n t)"),
                    0.0, OP.mult, OP.add)

                # hc into b_q (dead), pair-tree into h_q (dead), acc into yac
                nc.vector.tensor_tensor(b_q[:], h_q[:], Cq[q][:], OP.mult)
                nc.vector.tensor_tensor(h_q[:, 0:2, :], b_q[:, 0:2, :],
                                        b_q[:, 2:4, :], OP.add)
                nc.vector.tensor_tensor(h_q[:, 2, :], h_q[:, 0, :],
                                        h_q[:, 1, :], OP.add)
                nc.vector.tensor_tensor(yac[i][:], yac[i][:], h_q[:, 2, :], OP.add)

            nc.sync.dma_start(ybounce[i][:], yac[i][:])
            nc.gpsimd.collective_compute(
                "AllGather", OP.bypass,
                replica_groups=[[0, 1, 2, 3], [4, 5, 6, 7]],
                ins=[ybounce[i][:].opt()], outs=[ygather[i][:].opt()])
            if i + 1 < NDT:
                emit_front(i + 1)

        # ---- merge across directions (unpermute each slot, add)
        yc = []
        for i in range(NDT):
            sl = [tmp(tag="tmp") for _ in range(K)]
            for k in range(K):
                nc.sync.dma_start(sl[k][:], ygather[i][k * 128:(k + 1) * 128, :])
            r1 = sl[1][:].rearrange("p (w h) -> p h w", w=32)
            r2 = sl[2][:, ::-1]
            r3 = sl[3][:, ::-1].rearrange("p (w h) -> p h w", w=32)
            t01 = spool.tile([128, L], BF16, tag="sc1", name="t01")
            nc.vector.tensor_tensor(t01[:].rearrange("p (h w) -> p h w", h=32),
                                    sl[0][:].rearrange("p (h w) -> p h w", h=32),
                                    r1, OP.add)
            t23 = spool.tile([128, L], BF16, tag="sc2", name="t23")
            nc.vector.tensor_tensor(t23[:].rearrange("p (h w) -> p h w", h=32),
                                    r2.rearrange("p (h w) -> p h w", h=32),
                                    r3, OP.add)
            yci = rpool.tile([128, L], BF16, tag=f"xp_pad{i}", name=f"yc{i}")
            nc.vector.tensor_tensor(yci[:], t01[:], t23[:], OP.add)
            yc.append(yci)

        # ---- z projection + silu (zsil reuses delta's buffers)
        xTc0 = spool.tile([128, 2, L // 2], BF16, tag="xtc0", name="xTc0")
        nc.sync.dma_start(xTc0[:], xTc[0:128, :].rearrange("p (s t) -> p s t", s=2))
        xTc1 = spool.tile([64, 2, L // 2], BF16, tag="xtc1", name="xTc1")
        nc.sync.dma_start(xTc1[:], xTc[128:192, :].rearrange("p (s t) -> p s t", s=2))
        zsil = []
        for i in range(NDT):
            zsi = rpool.tile([128, L], BF16, tag=f"delta{i}", name=f"zsil{i}")
            for half in range(2):
                ps = proj_mm(O_WZ, i, xTc0[:, half, :], xTc1[:, half, :])
                nc.scalar.copy(zsi[:, half * 512:(half + 1) * 512], ps[:])
            sg = silu_sigmoid(zsi[:], 0.0)
            nc.vector.tensor_tensor(zsi[:], zsi[:], sg[:], OP.mult)
            zsil.append(zsi)

        # ---- LayerNorm stats (ones-matmul partition reduction, bf16 in, fp32 acc)
        ysq = []
        for i in range(NDT):
            sq = rpool.tile([128, L], BF16, tag=f"u{i}", name=f"ysq{i}")
            nc.scalar.activation(sq[:], yc[i][:], AF.Square)
            ysq.append(sq)
        psmu = ps_bc.tile([128, 1024], F32, tag="pn", name="psmu")
        psms = ps_bc.tile([128, 1024], F32, tag="pn", name="psms")
        for half in range(2):
            hs = slice(half * 512, (half + 1) * 512)
            for i in range(NDT):
                nc.tensor.matmul(psmu[:, hs], wA[:, O_ONES:O_ONES + 128],
                                 yc[i][:, hs], start=(i == 0), stop=(i == NDT - 1))
            for i in range(NDT):
                nc.tensor.matmul(psms[:, hs], wA[:, O_ONES:O_ONES + 128],
                                 ysq[i][:, hs], start=(i == 0), stop=(i == NDT - 1))
        mu_sb = spool.tile([128, L], F32, tag="mu", name="mu")
        nc.scalar.mul(mu_sb[:], psmu[:], 1.0 / DIN)
        musq = spool.tile([128, L], F32, tag="musq", name="musq")
        nc.scalar.activation(musq[:], mu_sb[:], AF.Square)
        ms_sb = spool.tile([128, L], F32, tag="vart", name="ms")
        nc.scalar.mul(ms_sb[:], psms[:], 1.0 / DIN)
        vart = spool.tile([128, L], F32, tag="d1", name="vart")
        nc.vector.tensor_tensor(vart[:], ms_sb[:], musq[:], OP.subtract)
        lnv = spool.tile([128, L], F32, tag="musq", name="lnv")
        nc.scalar.activation(lnv[:], vart[:], AF.Ln, bias=wF[:, F_EPS:F_EPS + 1])
        inv = spool.tile([128, L], F32, tag="vart", name="inv")
        nc.scalar.activation(inv[:], lnv[:], AF.Exp, scale=-0.5)

        # ---- normalize + gate + out projection
        yg = []
        for i in range(NDT):
            d1 = spool.tile([128, L], F32, tag="d1", name=f"d1_{i}")
            nc.vector.tensor_tensor(d1[:], yc[i][:], mu_sb[:], OP.subtract)
            d2 = spool.tile([128, L], F32, tag="d2", name=f"d2_{i}")
            nc.vector.tensor_tensor(d2[:], d1[:], inv[:], OP.mult)
            d3 = spool.tile([128, L], BF16, tag="sc1", name=f"d3_{i}")
            nc.scalar.activation(d3[:], d2[:], AF.Identity,
                                 bias=wF[:, F_B + i:F_B + i + 1],
                                 scale=wF[:, F_G + i:F_G + i + 1])
            ygi = rpool.tile([128, L], BF16, tag=f"yac{i}", name=f"yg{i}")
            nc.vector.tensor_tensor(ygi[:], d3[:], zsil[i][:], OP.mult)
            yg.append(ygi)

        for c in range(8):
            pso = ps_bc.tile([128, DM], F32, tag="pm", name="pso")
            for i in range(NDT):
                nc.tensor.matmul(pso[:], yg[i][:, c * 128:(c + 1) * 128],
                                 wA[:, O_WOUT + i * DM:O_WOUT + (i + 1) * DM],
                                 start=(i == 0), stop=(i == NDT - 1))
            ob = obpool.tile([128, DM], F32, tag="ob", name="ob")
            nc.scalar.copy(ob[:], pso[:])
            nc.sync.dma_start(out_d[c * 128:(c + 1) * 128, :], ob[:])

    nc.compile()
    return nc


def _prep_maps(inputs):
    x = np.asarray(inputs["x"], np.float32)
    x_cross = np.asarray(inputs["x_cross"], np.float32)
    in_proj_w = np.asarray(inputs["in_proj_w"], np.float32)
    in_proj_cross_w = np.asarray(inputs["in_proj_cross_w"], np.float32)
    conv_w = np.asarray(inputs["conv_w"], np.float32)
    conv_b = np.asarray(inputs["conv_b"], np.float32)
    x_proj_weight = np.asarray(inputs["x_proj_weight"], np.float32)
    dt_projs_weight = np.asarray(inputs["dt_projs_weight"], np.float32)
    dt_projs_bias = np.asarray(inputs["dt_projs_bias"], np.float32)
    A_logs = np.asarray(inputs["A_logs"], np.float32)
    Ds = np.asarray(inputs["Ds"], np.float32)
    out_norm_g = np.asarray(inputs["out_norm_g"], np.float32)
    out_norm_b = np.asarray(inputs["out_norm_b"], np.float32)
    out_proj_w = np.asarray(inputs["out_proj_w"], np.float32)

    W_xp = in_proj_w[:DIN]
    W_z = in_proj_w[DIN:2 * DIN]
    A_full = (-np.exp(A_logs)).reshape(K, DIN, N)
    Ds_k = Ds.reshape(K, DIN)

    def fold3(v):  # [384] -> [128, 3]
        return np.ascontiguousarray(v.reshape(NDT, 128).T)

    wxpT = np.ascontiguousarray(W_xp.T)        # [192, 384]
    wzT = np.ascontiguousarray(W_z.T)
    outT = np.ascontiguousarray(
        out_proj_w.T.reshape(NDT, 128, DM).transpose(1, 0, 2).reshape(128, NDT * DM))

    blobF0 = np.zeros((128, FBLOB), np.float32)
    blobF0[:, F_CONVB:F_CONVB + 3] = fold3(conv_b)
    blobF0[:, F_CONVBN:F_CONVBN + 3] = -fold3(conv_b)
    blobF0[:, F_G:F_G + 3] = fold3(out_norm_g)
    blobF0[:, F_B:F_B + 3] = fold3(out_norm_b)
    blobF0[:, F_EPS] = 1e-5

    in_maps = []
    for c in range(NCORES):
        b, k = c // 4, c % 4
        p = _perm(k)
        xb = x[b].reshape(L, DM)
        xcb = x_cross[b].reshape(L, DM)
        w = conv_w[:, 0]  # [384, 3, 3]
        if k == 0:
            wk = w
        elif k == 1:
            wk = w.transpose(0, 2, 1)
        elif k == 2:
            wk = w[:, ::-1, ::-1]
        else:
            wk = w.transpose(0, 2, 1)[:, ::-1, ::-1]
        wconv = np.zeros((128, 9 * NDT * 128), np.float32)
        for j in range(9):
            for i in range(NDT):
                m = j * NDT + i
                dgv = np.ascontiguousarray(wk[i * 128:(i + 1) * 128, j // 3, j % 3])
                wconv[:, m * 128:m * 128 + 128] = np.diag(dgv)

        # fold x_cross projection into x_dbl and dt weights
        xp_w = x_proj_weight[k]                      # [44, 384]
        W_bc = xp_w[R:R + 2 * N] @ in_proj_cross_w   # [32, 192]
        W_dd = (dt_projs_weight[k] @ xp_w[0:R]) @ in_proj_cross_w  # [384, 192]
        wbcT = np.zeros((192, 64), np.float32)
        wbcT[:, 0:N] = W_bc[0:N].T
        wbcT[:, 32:32 + N] = W_bc[N:2 * N].T
        wddT = np.ascontiguousarray(W_dd.T)          # [192, 384]
        Am = np.ascontiguousarray(
            A_full[k].reshape(NDT, 128, N).transpose(1, 0, 2).reshape(128, NDT * N))

        blobA = np.zeros((128, ABLOB), np.float32)
        blobA[:, O_WXP:O_WXP + 384] = wxpT[0:128]
        blobA[:, O_WZ:O_WZ + 384] = wzT[0:128]
        blobA[:, O_WDD:O_WDD + 384] = wddT[0:128]
        blobA[:, O_CONV:O_CONV + 3456] = wconv
        blobA[:, O_XBC:O_XBC + 64] = wbcT[0:128]
        blobA[:, O_WOUT:O_WOUT + 576] = outT
        blobA[:, O_ONES:O_ONES + 128] = 1.0

        blobB = np.zeros((64, BBLOB), np.float32)
        blobB[:, 0:384] = wxpT[128:192]
        blobB[:, 384:768] = wzT[128:192]
        blobB[:, 768:1152] = wddT[128:192]
        blobB[:, 1152:1216] = wbcT[128:192]

        bF = blobF0.copy()
        bF[:, F_DTB:F_DTB + 3] = fold3(dt_projs_bias[k])
        bF[:, F_A:F_A + NDT * N] = Am
        bF[:, F_D:F_D + 3] = fold3(Ds_k[k])

        xT = np.ascontiguousarray(xb[p].T)
        xcT = np.ascontiguousarray(xcb[p].T)
        m = {
            "wblobA": blobA.astype(BF),
            "wblobB": blobB.astype(BF),
            "wblobF": bF,
            "xin": np.concatenate([xT, xcT], axis=1).astype(BF),
            "xTc": np.ascontiguousarray(xb.T).astype(BF),
        }
        in_maps.append(m)
    return in_maps


def kernel(**inputs):
    if "nc" not in _cache:
        _cache["nc"] = _build_nc()
    nc = _cache["nc"]
    in_maps = _prep_maps(inputs)
    res = run_bass_kernel_spmd(nc, in_maps, core_ids=list(range(NCORES)))
    out = np.zeros((B_, L, DM), np.float32)
    out[0] = res.results[0]["out"]
    out[1] = res.results[4]["out"]
    return out.reshape(B_, HH, WW, DM)


# revision 35
# speedup vs baseline: 2.0227x; 1.0006x over previous
"""CSS2D (cross selective-scan 2D) Trainium2 kernel.

Sharding: 8 cores = batch(2) x scan-direction(4). Each core runs the full
pipeline for its (b, k) in the direction's own time order; direction
permutations are applied host-side to the inputs (and to the depthwise-conv
taps, which commute with grid transpose/reversal), so all 8 cores execute one
uniform SPMD program. The 4-direction merge is a per-dtile bf16 AllGather
within each b-group (overlapped with the scan of the next dtile) followed by
on-chip unpermute-and-add, LayerNorm, gating and the output projection
(computed redundantly per group; core 4b's output is used).

Key optimizations vs the straightforward mapping:
- all matmuls in bf16 (PE fp32 is 4 cycles/row vs 1 for bf16)
- the x_cross projection is folded host-side into the x_dbl / dt-projection
  weights (x_dbl = (W_k W_xc) @ x_cross), removing a full [384,192] GEMM
- scan elementwise chain in bf16 (DVE 2x mode, quad-grouped states); scan
  internal state fp32
- B/C broadcast to 128 partitions via stride-0 DRAM-bounce DMAs
- silu expressed through the exp/ln activation table (sigma(x) =
  exp(-ln(1+exp(-x)))) so the scalar engine never switches tables
- weights ship as three packed blobs (one DMA each)
"""
import numpy as np
import ml_dtypes
from contextlib import ExitStack

import concourse.bacc as bacc
import concourse.bass as bass
import concourse.mybir as mybir
import concourse.tile as tile
from concourse.bass_utils import run_bass_kernel_spmd

F32 = mybir.dt.float32
BF16 = mybir.dt.bfloat16
AF = mybir.ActivationFunctionType
OP = mybir.AluOpType

B_, HH, WW = 2, 32, 32
L = HH * WW                    # 1024
DM, DIN, N, R, K = 192, 384, 16, 12, 4
NDT = DIN // 128               # 3 d-tiles
QN = 4                         # states per scan quad
NQ = N // QN                   # 4 quads
NCORES = 8
PAD = 34 * 34                  # padded conv plane
BF = ml_dtypes.bfloat16

# blob A (bf16, [128, .]) column offsets
O_WXP, O_WZ, O_WDD = 0, 384, 768
O_CONV = 1152                  # 9*3*128 = 3456
O_XBC = 4608                   # 64 cols (B rows 0:16, C rows 32:48)
O_WOUT = 4672                  # 3*192 = 576
O_ONES = 5248
ABLOB = 5376
# blob B (bf16, [64, .]): wxpT1 @0, wzT1 @384, wddT1 @768, xbc1 @1152
BBLOB = 1216
# blob F (f32, [128, .]) column offsets
F_CONVB, F_CONVBN, F_DTB, F_A, F_D, F_G, F_B, F_EPS = 0, 3, 6, 9, 57, 60, 63, 66
FBLOB = 67

_cache = {}


class _PinnedActBacc(bacc.Bacc):
    """Restrict activation-table selection to the single table that covers
    every function this kernel uses (exp, ln, identity, square, copy), so the
    scalar engine loads its table once instead of thrashing between the
    first-match tables for exp and ln."""

    def insert_act_table_loads(self):
        import concourse.bacc as _bacc_mod
        import bass_rust as _bass_rust
        from concourse.hw_specs import get_activation_tables
        has_activation = any(
            isinstance(i, mybir.InstActivation)
            for b in self.main_func.blocks
            for i in b.instructions
        )
        if not has_activation:
            return
        mine = {AF.Exp, AF.Ln, AF.Identity, AF.Copy, AF.Square}
        tables = []
        seen = False
        for k, v in get_activation_tables(self.m.arch).items():
            if k == "natural_log_exp_and_others":
                seen = True
                assert mine <= v, f"{k} missing {mine - v}"
                tables.append((k, v))
            else:
                tables.append((k, v - mine))
        assert seen, "natural_log_exp_and_others table missing"
        _bass_rust.insert_act_table_loads(self, tables)


def _perm(k):
    t = np.arange(L)
    if k == 0:
        return t
    if k == 1:
        return (t % 32) * 32 + t // 32
    if k == 2:
        return 1023 - t
    return _perm(1)[1023 - t]


def _build_nc():
    nc = _PinnedActBacc(None, target_bir_lowering=False)

    wblobA = nc.declare_dram_parameter("wblobA", [128, ABLOB], BF16, isOutput=False)
    wblobB = nc.declare_dram_parameter("wblobB", [64, BBLOB], BF16, isOutput=False)
    wblobF = nc.declare_dram_parameter("wblobF", [128, FBLOB], F32, isOutput=False)
    xin = nc.declare_dram_parameter("xin", [192, 2 * L], BF16, isOutput=False)
    xTc = nc.declare_dram_parameter("xTc", [192, L], BF16, isOutput=False)
    out_d = nc.declare_dram_parameter("out", [L, DM], F32, isOutput=True)

    with ExitStack() as ctx:
        tc = ctx.enter_context(tile.TileContext(nc))
        wpool = ctx.enter_context(tc.tile_pool(name="w", bufs=1))
        rpool = ctx.enter_context(tc.tile_pool(name="r", bufs=1))
        tpool = ctx.enter_context(tc.tile_pool(name="t", bufs=2))
        iopool = ctx.enter_context(tc.tile_pool(name="io", bufs=4))
        bcpool = ctx.enter_context(tc.tile_pool(name="bcp", bufs=1))
        apool = ctx.enter_context(tc.tile_pool(name="a", bufs=2))
        hpool = ctx.enter_context(tc.tile_pool(name="h", bufs=1))
        spool = ctx.enter_context(tc.tile_pool(name="s", bufs=1))
        obpool = ctx.enter_context(tc.tile_pool(name="obp", bufs=2))
        ps_bc = ctx.enter_context(tc.tile_pool(name="psc", bufs=2, space="PSUM"))
        dram = ctx.enter_context(tc.tile_pool(name="dram", bufs=1, space="DRAM"))

        def tmp(shape=(128, L), tag="tmp", dt=BF16):
            pool = iopool if tag == "tmp" else tpool
            return pool.tile(list(shape), dt, tag=tag, name=tag)

        # ---- weight blobs (one DMA each)
        wA = wpool.tile([128, ABLOB], BF16, tag="wA", name="wA")
        nc.sync.dma_start(wA[:], wblobA[:, :])
        wB = wpool.tile([64, BBLOB], BF16, tag="wB", name="wB")
        nc.sync.dma_start(wB[:], wblobB[:, :])
        wF = wpool.tile([128, FBLOB], F32, tag="wF", name="wF")
        nc.sync.dma_start(wF[:], wblobF[:, :])

        # ---- residents
        xp_pad = [rpool.tile([128, PAD], BF16, tag=f"xp_pad{i}", name=f"xp_pad{i}")
                  for i in range(NDT)]
        delta = [rpool.tile([128, L], BF16, tag=f"delta{i}", name=f"delta{i}")
                 for i in range(NDT)]
        uu = [rpool.tile([128, L], BF16, tag=f"u{i}", name=f"u{i}")
              for i in range(NDT)]
        yac = [rpool.tile([128, L], BF16, tag=f"yac{i}", name=f"yac{i}")
               for i in range(NDT)]
        xdblB = rpool.tile([N, L], BF16, tag="xdblB", name="xdblB")
        xdblC = rpool.tile([N, L], BF16, tag="xdblC", name="xdblC")
        # B/C broadcast tiles, bf16, quad-grouped [128, QN, L]
        Bq = [bcpool.tile([128, QN, L], BF16, tag=f"Bq{q}", name=f"Bq{q}")
              for q in range(NQ)]
        Cq = [bcpool.tile([128, QN, L], BF16, tag=f"Cq{q}", name=f"Cq{q}")
              for q in range(NQ)]

        for i in range(NDT):
            nc.vector.memset(xp_pad[i][:], 0.0)

        # ---- input loads (xT/xcT packed in one param)
        xin0 = rpool.tile([128, 2, L], BF16, tag="xin0", name="xin0")
        nc.sync.dma_start(xin0[:], xin[0:128, :].rearrange("p (s t) -> p s t", s=2))
        xin1 = rpool.tile([64, 2, L], BF16, tag="xin1", name="xin1")
        nc.sync.dma_start(xin1[:], xin[128:192, :].rearrange("p (s t) -> p s t", s=2))

        _WB_BASE = {O_WXP: 0, O_WZ: 384, O_WDD: 768}

        def proj_mm(wc0, i, rhs0, rhs1):
            ps = ps_bc.tile([128, 512], F32, tag="pm", name="pm")
            c0 = wc0 + i * 128
            b0 = _WB_BASE[wc0] + i * 128
            nc.tensor.matmul(ps[:], wA[:, c0:c0 + 128], rhs0,
                             start=True, stop=False)
            nc.tensor.matmul(ps[:], wB[:, b0:b0 + 128], rhs1,
                             start=False, stop=True)
            return ps

        # ---- x_dbl = (W_k W_xc) @ x_cross^T : B rows 0:16, C rows 16:32
        for half in range(2):
            psx = ps_bc.tile([128, 512], F32, tag="pm", name="px")
            hs = slice(half * 512, (half + 1) * 512)
            nc.tensor.matmul(psx[0:64, :], wA[:, O_XBC:O_XBC + 64],
                             xin0[:, 1, hs], start=True, stop=False)
            nc.tensor.matmul(psx[0:64, :], wB[:, 1152:1152 + 64],
                             xin1[:, 1, hs], start=False, stop=True)
            nc.scalar.copy(xdblB[:, hs], psx[0:N, :])
            nc.scalar.copy(xdblC[:, hs], psx[32:32 + N, :])

        # ---- B/C broadcast to 128 partitions via DRAM-bounce stride-0 DMAs,
        # one DMA per (quad, B/C). Interleaved with the dtile-0 scan below.
        bcB = dram.tile([N, L], BF16, tag="bcB", name="bcB")
        bcC = dram.tile([N, L], BF16, tag="bcC", name="bcC")
        nc.sync.dma_start(bcB[:], xdblB[:])
        nc.sync.dma_start(bcC[:], xdblC[:])

        def emit_bc(q):
            for src, dst in ((bcB, Bq[q]), (bcC, Cq[q])):
                s_ap, d_ap = bass.broadcast_tensor_aps(
                    src[None, q * QN:(q + 1) * QN, :], dst[:])
                nc.sync.dma_start(d_ap, s_ap)

        def silu_sigmoid(src_ap, biasn):
            """sigma(src + bias) = exp(-ln(1 + exp(-(src+bias)))) on the
            exp/ln table. biasn is an AP holding the NEGATED bias (or 0.0)."""
            e1 = tmp(tag="sg")
            nc.scalar.activation(e1[:], src_ap, AF.Exp, scale=-1.0, bias=biasn)
            l1 = tmp(tag="sg")
            nc.scalar.activation(l1[:], e1[:], AF.Ln, bias=1.0)
            sg = tmp(tag="sg")
            nc.scalar.activation(sg[:], l1[:], AF.Exp, scale=-1.0)
            return sg

        # ---- per-dtile front-end: xp proj -> conv+silu -> dt -> delta/u/yac
        xh = [xp_pad[i][:, 0:L] for i in range(NDT)]

        def emit_front(i):
            for half in range(2):
                ps = proj_mm(O_WXP, i, xin0[:, 0, half * 512:(half + 1) * 512],
                             xin1[:, 0, half * 512:(half + 1) * 512])
                dst = xp_pad[i][:].rearrange("p (h w) -> p h w", h=34)
                h0 = 1 + 16 * half
                nc.scalar.copy(dst[:, h0:h0 + 16, 1:33],
                               ps[:].rearrange("p (h w) -> p h w", h=16))
            psc = ps_bc.tile([128, 1024], F32, tag="pn", name="psconv")
            pad3 = xp_pad[i][:].rearrange("p (h w) -> p h w", h=34)
            for j in range(9):
                oh, ow = j // 3, j % 3
                c0 = O_CONV + (j * NDT + i) * 128
                wsl = wA[:, c0:c0 + 128]
                for half in range(2):
                    h0 = oh + 16 * half
                    win = pad3[:, h0:h0 + 16, ow:ow + 32]
                    nc.tensor.matmul(psc[:, half * 512:(half + 1) * 512],
                                     wsl, win, start=(j == 0), stop=(j == 8))
            # conv out + bias in SBUF; silu via exp/ln table
            xpc = tmp(tag="pa")
            nc.scalar.activation(xpc[:], psc[:], AF.Identity,
                                 bias=wF[:, F_CONVB + i:F_CONVB + i + 1])
            sg = silu_sigmoid(psc[:], wF[:, F_CONVBN + i:F_CONVBN + i + 1])
            nc.vector.tensor_tensor(xh[i], xpc[:], sg[:], OP.mult)
            # delta = softplus((W_dd @ xc) + dtb) via exp/ln
            psd = ps_bc.tile([128, 1024], F32, tag="pn", name="psd")
            for half in range(2):
                ps = proj_mm(O_WDD, i, xin0[:, 1, half * 512:(half + 1) * 512],
                             xin1[:, 1, half * 512:(half + 1) * 512])
                nc.scalar.copy(psd[:, half * 512:(half + 1) * 512], ps[:])
            et = tmp(tag="pa")
            nc.scalar.activation(et[:], psd[:], AF.Exp,
                                 bias=wF[:, F_DTB + i:F_DTB + i + 1], scale=1.0)
            nc.scalar.activation(delta[i][:], et[:], AF.Ln, bias=1.0)
            nc.vector.tensor_tensor(uu[i][:], delta[i][:], xh[i], OP.mult)
            nc.vector.tensor_scalar_mul(yac[i][:], xh[i], wF[:, F_D + i:F_D + i + 1])

        # ---- scan: dtile outer (per-dtile AllGather overlaps next dtile)
        ybounce = [dram.tile([128, L], BF16, tag=f"ybounce{i}", name=f"ybounce{i}")
                   for i in range(NDT)]
        ygather = [dram.tile([K * 128, L], BF16, tag=f"ygather{i}", name=f"ygather{i}")
                   for i in range(NDT)]
        emit_front(0)
        for i in range(NDT):
            for q in range(NQ):
                if i == 0:
                    emit_bc(q)
                a_q = apool.tile([128, QN, L], BF16, tag="a_q", name=f"a{i}{q}")
                for j in range(QN):
                    n = q * QN + j
                    nc.scalar.activation(a_q[:, j, :], delta[i][:], AF.Exp,
                                         scale=wF[:, F_A + i * N + n:F_A + i * N + n + 1])
                nc.vector.memset(a_q[:, :, 0:1], 0.0)

                b_q = hpool.tile([128, QN, L], BF16, tag="b_q", name=f"b{i}{q}")
                a0, a1 = bass.broadcast_tensor_aps(uu[i][:, None, :], Bq[q][:])
                nc.vector.tensor_tensor(b_q[:], a0, a1, OP.mult)

                h_q = hpool.tile([128, QN, L], BF16, tag="h_q", name=f"h{i}{q}")
                nc.vector.tensor_tensor_scan(
                    h_q[:].rearrange("p n t -> p (n t)"),
                    a_q[:].rearrange("p n t -> p (n t)"),
                    b_q[:].rearrange("p n t -> p (n t)"),
                    0.0, OP.mult, OP.add)

                # hc into b_q (dead), pair-tree into h_q (dead), acc into yac
                nc.vector.tensor_tensor(b_q[:], h_q[:], Cq[q][:], OP.mult)
                nc.vector.tensor_tensor(h_q[:, 0:2, :], b_q[:, 0:2, :],
                                        b_q[:, 2:4, :], OP.add)
                nc.vector.tensor_tensor(h_q[:, 2, :], h_q[:, 0, :],
                                        h_q[:, 1, :], OP.add)
                nc.vector.tensor_tensor(yac[i][:], yac[i][:], h_q[:, 2, :], OP.add)

            nc.sync.dma_start(ybounce[i][:], yac[i][:])
            nc.gpsimd.collective_compute(
                "AllGather", OP.bypass,
                replica_groups=[[0, 1, 2, 3], [4, 5, 6, 7]],
                ins=[ybounce[i][:].opt()], outs=[ygather[i][:].opt()])
            if i + 1 < NDT:
                emit_front(i + 1)

        # ---- merge across directions (unpermute each slot, add)
        yc = []
        for i in range(NDT):
            sl = [tmp(tag="tmp") for _ in range(K)]
            for k in range(K):
                nc.sync.dma_start(sl[k][:], ygather[i][k * 128:(k + 1) * 128, :])
            r1 = sl[1][:].rearrange("p (w h) -> p h w", w=32)
            r2 = sl[2][:, ::-1]
            r3 = sl[3][:, ::-1].rearrange("p (w h) -> p h w", w=32)
            t01 = spool.tile([128, L], BF16, tag="sc1", name="t01")
            nc.vector.tensor_tensor(t01[:].rearrange("p (h w) -> p h w", h=32),
                                    sl[0][:].rearrange("p (h w) -> p h w", h=32),
                                    r1, OP.add)
            t23 = spool.tile([128, L], BF16, tag="sc2", name="t23")
            nc.vector.tensor_tensor(t23[:].rearrange("p (h w) -> p h w", h=32),
                                    r2.rearrange("p (h w) -> p h w", h=32),
                                    r3, OP.add)
            yci = rpool.tile([128, L], BF16, tag=f"xp_pad{i}", name=f"yc{i}")
            nc.vector.tensor_tensor(yci[:], t01[:], t23[:], OP.add)
            yc.append(yci)

        # ---- z projection + silu (zsil reuses delta's buffers)
        xTc0 = spool.tile([128, 2, L // 2], BF16, tag="xtc0", name="xTc0")
        nc.sync.dma_start(xTc0[:], xTc[0:128, :].rearrange("p (s t) -> p s t", s=2))
        xTc1 = spool.tile([64, 2, L // 2], BF16, tag="xtc1", name="xTc1")
        nc.sync.dma_start(xTc1[:], xTc[128:192, :].rearrange("p (s t) -> p s t", s=2))
        zsil = []
        for i in range(NDT):
            zsi = rpool.tile([128, L], BF16, tag=f"delta{i}", name=f"zsil{i}")
            for half in range(2):
                ps = proj_mm(O_WZ, i, xTc0[:, half, :], xTc1[:, half, :])
                nc.scalar.copy(zsi[:, half * 512:(half + 1) * 512], ps[:])
            sg = silu_sigmoid(zsi[:], 0.0)
            nc.vector.tensor_tensor(zsi[:], zsi[:], sg[:], OP.mult)
            zsil.append(zsi)

        # ---- LayerNorm stats (ones-matmul partition reduction, bf16 in, fp32 acc)
        ysq = []
        for i in range(NDT):
            sq = rpool.tile([128, L], BF16, tag=f"u{i}", name=f"ysq{i}")
            nc.scalar.activation(sq[:], yc[i][:], AF.Square)
            ysq.append(sq)
        psmu = ps_bc.tile([128, 1024], F32, tag="pn", name="psmu")
        psms = ps_bc.tile([128, 1024], F32, tag="pn", name="psms")
        for half in range(2):
            hs = slice(half * 512, (half + 1) * 512)
            for i in range(NDT):
                nc.tensor.matmul(psmu[:, hs], wA[:, O_ONES:O_ONES + 128],
                                 yc[i][:, hs], start=(i == 0), stop=(i == NDT - 1))
            for i in range(NDT):
                nc.tensor.matmul(psms[:, hs], wA[:, O_ONES:O_ONES + 128],
                                 ysq[i][:, hs], start=(i == 0), stop=(i == NDT - 1))
        mu_sb = spool.tile([128, L], F32, tag="mu", name="mu")
        nc.scalar.mul(mu_sb[:], psmu[:], 1.0 / DIN)
        musq = spool.tile([128, L], F32, tag="musq", name="musq")
        nc.scalar.activation(musq[:], mu_sb[:], AF.Square)
        ms_sb = spool.tile([128, L], F32, tag="vart", name="ms")
        nc.scalar.mul(ms_sb[:], psms[:], 1.0 / DIN)
        vart = spool.tile([128, L], F32, tag="d1", name="vart")
        nc.vector.tensor_tensor(vart[:], ms_sb[:], musq[:], OP.subtract)
        lnv = spool.tile([128, L], F32, tag="musq", name="lnv")
        nc.scalar.activation(lnv[:], vart[:], AF.Ln, bias=wF[:, F_EPS:F_EPS + 1])
        inv = spool.tile([128, L], F32, tag="vart", name="inv")
        nc.scalar.activation(inv[:], lnv[:], AF.Exp, scale=-0.5)

        # ---- normalize + gate + out projection
        yg = []
        for i in range(NDT):
            d1 = spool.tile([128, L], F32, tag="d1", name=f"d1_{i}")
            nc.vector.tensor_tensor(d1[:], yc[i][:], mu_sb[:], OP.subtract)
            d2 = spool.tile([128, L], F32, tag="d2", name=f"d2_{i}")
            nc.vector.tensor_tensor(d2[:], d1[:], inv[:], OP.mult)
            d3 = spool.tile([128, L], BF16, tag="sc1", name=f"d3_{i}")
            nc.scalar.activation(d3[:], d2[:], AF.Identity,
                                 bias=wF[:, F_B + i:F_B + i + 1],
                                 scale=wF[:, F_G + i:F_G + i + 1])
            ygi = rpool.tile([128, L], BF16, tag=f"yac{i}", name=f"yg{i}")
            nc.vector.tensor_tensor(ygi[:], d3[:], zsil[i][:], OP.mult)
            yg.append(ygi)

        for c in range(8):
            pso = ps_bc.tile([128, DM], F32, tag="pm", name="pso")
            for i in range(NDT):
                nc.tensor.matmul(pso[:], yg[i][:, c * 128:(c + 1) * 128],
                                 wA[:, O_WOUT + i * DM:O_WOUT + (i + 1) * DM],
                                 start=(i == 0), stop=(i == NDT - 1))
            ob = obpool.tile([128, DM], F32, tag="ob", name="ob")
            nc.scalar.copy(ob[:], pso[:])
            nc.sync.dma_start(out_d[c * 128:(c + 1) * 128, :], ob[:])

    nc.compile()
    return nc


def _prep_maps(inputs):
    x = np.asarray(inputs["x"], np.float32)
    x_cross = np.asarray(inputs["x_cross"], np.float32)
    in_proj_w = np.asarray(inputs["in_proj_w"], np.float32)
    in_proj_cross_w = np.asarray(inputs["in_proj_cross_w"], np.float32)
    conv_w = np.asarray(inputs["conv_w"], np.float32)
    conv_b = np.asarray(inputs["conv_b"], np.float32)
    x_proj_weight = np.asarray(inputs["x_proj_weight"], np.float32)
    dt_projs_weight = np.asarray(inputs["dt_projs_weight"], np.float32)
    dt_projs_bias = np.asarray(inputs["dt_projs_bias"], np.float32)
    A_logs = np.asarray(inputs["A_logs"], np.float32)
    Ds = np.asarray(inputs["Ds"], np.float32)
    out_norm_g = np.asarray(inputs["out_norm_g"], np.float32)
    out_norm_b = np.asarray(inputs["out_norm_b"], np.float32)
    out_proj_w = np.asarray(inputs["out_proj_w"], np.float32)

    W_xp = in_proj_w[:DIN]
    W_z = in_proj_w[DIN:2 * DIN]
    A_full = (-np.exp(A_logs)).reshape(K, DIN, N)
    Ds_k = Ds.reshape(K, DIN)

    def fold3(v):  # [384] -> [128, 3]
        return np.ascontiguousarray(v.reshape(NDT, 128).T)

    wxpT = np.ascontiguousarray(W_xp.T)        # [192, 384]
    wzT = np.ascontiguousarray(W_z.T)
    outT = np.ascontiguousarray(
        out_proj_w.T.reshape(NDT, 128, DM).transpose(1, 0, 2).reshape(128, NDT * DM))

    blobF0 = np.zeros((128, FBLOB), np.float32)
    blobF0[:, F_CONVB:F_CONVB + 3] = fold3(conv_b)
    blobF0[:, F_CONVBN:F_CONVBN + 3] = -fold3(conv_b)
    blobF0[:, F_G:F_G + 3] = fold3(out_norm_g)
    blobF0[:, F_B:F_B + 3] = fold3(out_norm_b)
    blobF0[:, F_EPS] = 1e-5

    in_maps = []
    for c in range(NCORES):
        b, k = c // 4, c % 4
        p = _perm(k)
        xb = x[b].reshape(L, DM)
        xcb = x_cross[b].reshape(L, DM)
        w = conv_w[:, 0]  # [384, 3, 3]
        if k == 0:
            wk = w
        elif k == 1:
            wk = w.transpose(0, 2, 1)
        elif k == 2:
            wk = w[:, ::-1, ::-1]
        else:
            wk = w.transpose(0, 2, 1)[:, ::-1, ::-1]
        wconv = np.zeros((128, 9 * NDT * 128), np.float32)
        for j in range(9):
            for i in range(NDT):
                m = j * NDT + i
                dgv = np.ascontiguousarray(wk[i * 128:(i + 1) * 128, j // 3, j % 3])
                wconv[:, m * 128:m * 128 + 128] = np.diag(dgv)

        # fold x_cross projection into x_dbl and dt weights
        xp_w = x_proj_weight[k]                      # [44, 384]
        W_bc = xp_w[R:R + 2 * N] @ in_proj_cross_w   # [32, 192]
        W_dd = (dt_projs_weight[k] @ xp_w[0:R]) @ in_proj_cross_w  # [384, 192]
        wbcT = np.zeros((192, 64), np.float32)
        wbcT[:, 0:N] = W_bc[0:N].T
        wbcT[:, 32:32 + N] = W_bc[N:2 * N].T
        wddT = np.ascontiguousarray(W_dd.T)          # [192, 384]
        Am = np.ascontiguousarray(
            A_full[k].reshape(NDT, 128, N).transpose(1, 0, 2).reshape(128, NDT * N))

        blobA = np.zeros((128, ABLOB), np.float32)
        blobA[:, O_WXP:O_WXP + 384] = wxpT[0:128]
        blobA[:, O_WZ:O_WZ + 384] = wzT[0:128]
        blobA[:, O_WDD:O_WDD + 384] = wddT[0:128]
        blobA[:, O_CONV:O_CONV + 3456] = wconv
        blobA[:, O_XBC:O_XBC + 64] = wbcT[0:128]
        blobA[:, O_WOUT:O_WOUT + 576] = outT
        blobA[:, O_ONES:O_ONES + 128] = 1.0

        blobB = np.zeros((64, BBLOB), np.float32)
        blobB[:, 0:384] = wxpT[128:192]
        blobB[:, 384:768] = wzT[128:192]
        blobB[:, 768:1152] = wddT[128:192]
        blobB[:, 1152:1216] = wbcT[128:192]

        bF = blobF0.copy()
        bF[:, F_DTB:F_DTB + 3] = fold3(dt_projs_bias[k])
        bF[:, F_A:F_A + NDT * N] = Am
        bF[:, F_D:F_D + 3] = fold3(Ds_k[k])

        xT = np.ascontiguousarray(xb[p].T)
        xcT = np.ascontiguousarray(xcb[p].T)
        m = {
            "wblobA": blobA.astype(BF),
            "wblobB": blobB.astype(BF),
            "wblobF": bF,
            "xin": np.concatenate([xT, xcT], axis=1).astype(BF),
            "xTc": np.ascontiguousarray(xb.T).astype(BF),
        }
        in_maps.append(m)
    return in_maps


def kernel(**inputs):
    if "nc" not in _cache:
        _cache["nc"] = _build_nc()
    nc = _cache["nc"]
    in_maps = _prep_maps(inputs)
    res = run_bass_kernel_spmd(nc, in_maps, core_ids=list(range(NCORES)))
    out = np.zeros((B_, L, DM), np.float32)
    out[0] = res.results[0]["out"]
    out[1] = res.results[4]["out"]
    return out.reshape(B_, HH, WW, DM)
